# revision 15
# baseline (speedup 1.0000x reference)
"""Llama attention layer (B=2, S=2048, D=2048, H=16, HD=128, RoPE, causal)
on 8 Trainium2 NeuronCores.

Sharding: core c -> (batch b = c//4, head group g = c%4 of 4 heads).
Each core computes q/k/v projections for its 512 columns of wq/wk/wv,
RoPE, causal attention for its 4 heads, and the out-projection against
its 512 rows of wo (a partial sum over head groups). The host sums the
4 partials per batch and stacks the 2 batches.

All device matmuls run in bf16 with fp32 PSUM accumulation. Softmax is
computed without max-subtraction (scores here are bounded ~|9|).

v2 changes over the first working version:
- V-projection runs k-outer / m-inner in two passes of 8 PSUM banks so
  the first matmuls start as soon as x chunk 0 lands (DMA cascade of
  interleaved wv/x chunk transfers) instead of ~16us in.
- Exact-causal trimming: score/AV matmuls and the exp only cover
  q >= key-chunk-start column ranges (53% of the S x S square instead
  of the 62.5% block-causal coverage), with a single shared [128,128]
  triangle mask.
- Softmax denominator: exp chunks are chain-accumulated on DVE in bf16
  and reduced by ONE ones-matmul per (block, head) instead of one per
  chunk pair (saves ~33k PE cycles/core).
- The final 512 q columns are processed as two 256/128-col blocks and
  each block's out-projection is deferred past the next block's first
  head, hiding the softmax-normalize latency and shortening the tail.
- Output is written as fp16 partials (halves output DMA bytes; host
  sums partials in fp32).
"""

import os
import sys

import numpy as np
import ml_dtypes

if "/opt/trn_rl_repo" not in sys.path:
    sys.path.insert(0, "/opt/trn_rl_repo")

import concourse.bass as bass  # noqa: E402
import concourse.mybir as mybir  # noqa: E402
import concourse.bacc as bacc  # noqa: E402
import concourse.tile as tile  # noqa: E402

BF16 = ml_dtypes.bfloat16

B, S, D, H = 2, 2048, 2048, 16
HD = D // H            # 128, head dim
G = 4                  # head groups (cores per batch)
NH = H // G            # 4 heads per core
DG = NH * HD           # 512, per-core head width
P = 128
KO = D // P            # 16 k-subtiles over D
NKT = S // P           # 16 key chunks of 128
NQT = S // 512         # 4 q tiles of 512
QT = 512
ROPE_THETA = 10000.0
SCALE = 1.0 / float(np.sqrt(HD))

# q-column blocks for the attention phase: (start, width).
# The last 512 columns are split so the final out-projections have less
# attention work serialized in front of them (shorter kernel tail).
BLOCKS = [(0, 512), (512, 512), (1024, 512), (1536, 256), (1792, 256)]

N_CORES = 8

_BUILT = None  # (nc,) cache


def build_module():
    fp32 = mybir.dt.float32
    fp16 = mybir.dt.float16
    bf16 = mybir.dt.bfloat16

    nc = bacc.Bacc("TRN2", target_bir_lowering=False, debug=False,
                   num_devices=N_CORES, num_swdge_queues=4)

    xT = nc.dram_tensor("xT", [P, KO, S], bf16, kind="ExternalInput")
    wq = nc.dram_tensor("wq", [P, KO, DG], bf16, kind="ExternalInput")
    wk = nc.dram_tensor("wk", [P, KO, DG], bf16, kind="ExternalInput")
    wv = nc.dram_tensor("wv", [P, KO, DG], bf16, kind="ExternalInput")
    wo = nc.dram_tensor("wo", [P, NH, D], bf16, kind="ExternalInput")
    cosT = nc.dram_tensor("cosT", [P, S], bf16, kind="ExternalInput")
    sinT = nc.dram_tensor("sinT", [P, S], bf16, kind="ExternalInput")
    maskD = nc.dram_tensor("maskD", [P, P], bf16, kind="ExternalInput")
    out = nc.dram_tensor("out", [P, NKT, D], fp16, kind="ExternalOutput")

    Exp = mybir.ActivationFunctionType.Exp

    with tile.TileContext(nc) as tc:
        with tc.tile_pool(name="const", bufs=1) as const, \
             tc.tile_pool(name="big", bufs=1) as big:
            ones = const.tile([P, P], bf16)
            nc.vector.memset(ones, 1.0)
            mask_sb = const.tile([P, P], bf16)
            # dummy exp so the ACT Exp table loads during the DMA prefix,
            # not at the first real exp in the attention phase
            warm = const.tile([1, 1], fp32)
            nc.scalar.activation(warm, ones[0:1, 0:1],
                                 mybir.ActivationFunctionType.Exp)

            qT_sb = big.tile([P, NH, S], bf16)   # per head: [HD, S]
            kT_sb = big.tile([P, NH, S], bf16)
            v_sb = big.tile([P, NKT, DG], bf16)  # [key%128, keychunk, dg]
            aoT_sb = big.tile([P, NH, S], bf16)  # attention out^T

            # ---------------- phase 1: projections + RoPE ----------------
            # PSUM is split into a 6-bank pool (ps1a) and a 2-bank pool
            # (ps1b). Everything phase 1 does lands on ps1a except the V
            # groups m6/7/14/15 and the LAST Q head-pair, which go to ps1b.
            # The attention-phase pools for scores/ps_o/sum reuse only
            # ps1a's banks, so their allocation waits on ps1a's release
            # (done as the last Q matmuls retire) and NOT on the final
            # rope drains that hold ps1b — those only gate the out-proj
            # pool, which is first needed several microseconds later.
            with tc.tile_pool(name="w_pool", bufs=1) as w_pool, \
                 tc.tile_pool(name="rope", bufs=4) as rope, \
                 tc.tile_pool(name="ps1a", bufs=1, space="PSUM") as ps1a, \
                 tc.tile_pool(name="ps1b", bufs=1, space="PSUM") as ps1b:
                # DMA cascade: per-k interleaved wv/x chunks so the V loop
                # (k-outer) can start as soon as chunk 0 lands.
                wv_sb = w_pool.tile([P, KO, DG], bf16)
                xT_sb = w_pool.tile([P, KO, S], bf16)
                for k in range(KO):
                    nc.sync.dma_start(wv_sb[:, k, :], wv.ap()[:, k, :])
                    nc.sync.dma_start(xT_sb[:, k, :], xT.ap()[:, k, :])
                wk_sb = w_pool.tile([P, KO, DG], bf16)
                nc.sync.dma_start(wk_sb, wk.ap())
                wq_sb = w_pool.tile([P, KO, DG], bf16)
                nc.sync.dma_start(wq_sb, wq.ap())
                # cos/sin live in the rope pool (NOT w_pool): their last
                # readers are the final rope muls, and the attention-phase
                # SBUF pools fit inside w_pool's zone alone, so keeping
                # cos/sin out of w_pool lets those pools allocate as soon
                # as the last Q matmul (w_pool's true last reader) retires.
                cos_sb = rope.tile([P, S], bf16, tag="cos", bufs=1)
                nc.sync.dma_start(cos_sb, cosT.ap())
                sin_sb = rope.tile([P, S], bf16, tag="sin", bufs=1)
                nc.sync.dma_start(sin_sb, sinT.ap())
                nc.sync.dma_start(mask_sb, maskD.ap())

                def p1tile(last):
                    if last:
                        return ps1b.tile([P, QT], fp32, tag="psvb",
                                         name="psvb", bufs=2)
                    return ps1a.tile([P, QT], fp32, tag="psva",
                                     name="psva", bufs=6)

                # V: [keys, dg] natural layout. k-outer over two passes of
                # 8 m-groups (8 PSUM banks) so each arriving (wv, x) chunk
                # pair is consumed by 8 matmuls immediately.
                for half in range(2):
                    ms = list(range(8 * half, 8 * half + 8))
                    pss = {}
                    for m in ms:
                        pss[m] = p1tile(m % 8 >= 6)
                    for k in range(KO):
                        for m in ms:
                            nc.tensor.matmul(
                                pss[m], xT_sb[:, k, m * P:(m + 1) * P],
                                wv_sb[:, k, :],
                                start=(k == 0), stop=(k == KO - 1))
                    for m in ms:
                        nc.scalar.copy(v_sb[:, m, :], pss[m])

                # K then Q: [HD, S] transposed layout + RoPE.
                # Heads processed in pairs; psum groups rotate the 6-deep
                # ps1a ring so RoPE of one pair overlaps the next matmuls.
                for which, w_sb, dstT in (("k", wk_sb, kT_sb), ("q", wq_sb, qT_sb)):
                    for nt2 in range(2 * NQT):
                        nt, hp = divmod(nt2, 2)
                        sl = slice(nt * QT, (nt + 1) * QT)
                        heads = (2 * hp, 2 * hp + 1)
                        last_pair = which == "q" and nt2 == 2 * NQT - 1
                        pss = {}
                        for h in heads:
                            pss[h] = p1tile(last_pair)
                        for k in range(KO):
                            for h in heads:
                                nc.tensor.matmul(
                                    pss[h], w_sb[:, k, h * HD:(h + 1) * HD],
                                    xT_sb[:, k, sl],
                                    start=(k == 0), stop=(k == KO - 1))
                        for h in heads:
                            ps = pss[h]
                            dst = dstT[:, h, sl]
                            # rope: dst = ps * cos + swap(ps) * sin_signed.
                            # The swapped reads must come from PSUM (the SB-SB
                            # same-base-partition rule forbids them on SBUF);
                            # the straight read goes via a parallel ACT copy so
                            # the psum bank drains fast.
                            tmp = rope.tile([P, QT], bf16, tag="tmp")
                            nc.vector.tensor_mul(tmp[0:64], ps[64:128],
                                                 sin_sb[0:64, sl])
                            nc.vector.tensor_mul(tmp[64:128], ps[0:64],
                                                 sin_sb[64:128, sl])
                            qb = rope.tile([P, QT], bf16, tag="qb")
                            nc.scalar.copy(qb, ps)
                            nc.vector.tensor_mul(dst, qb, cos_sb[:, sl])
                            nc.vector.tensor_add(dst, dst, tmp)

            # ---------------- phases 2+3 ----------------
            with tc.tile_pool(name="big2", bufs=1) as big2:
                wo_sb = big2.tile([P, NH, D], bf16)
                nc.sync.dma_start(wo_sb, wo.ap())

                with tc.tile_pool(name="ax_pool", bufs=10) as ax_pool, \
                     tc.tile_pool(name="accp", bufs=3) as accp, \
                     tc.tile_pool(name="ep", bufs=3) as ep, \
                     tc.tile_pool(name="stage", bufs=6) as stage, \
                     tc.tile_pool(name="ps2s", bufs=3, space="PSUM") as ps2s, \
                     tc.tile_pool(name="ps2o", bufs=2, space="PSUM") as ps2o, \
                     tc.tile_pool(name="ps2", bufs=1, space="PSUM") as ps2, \
                     tc.tile_pool(name="ps3", bufs=2, space="PSUM") as ps3:

                    drain_ct = 0

                    def outproj_qo(qo):
                        nonlocal drain_ct
                        for n in range(D // QT):
                            nsl = slice(n * QT, (n + 1) * QT)
                            ps = ps3.tile([P, QT], fp32, tag="ps_out")
                            for h in range(NH):
                                nc.tensor.matmul(
                                    ps, aoT_sb[:, h, qo * P:(qo + 1) * P],
                                    wo_sb[:, h, nsl],
                                    start=(h == 0), stop=(h == NH - 1))
                            ob = stage.tile([P, QT], fp16, tag="ob")
                            # alternate the psum drains across ACT/DVE
                            if drain_ct % 2 == 0:
                                nc.scalar.copy(ob, ps)
                            else:
                                nc.vector.tensor_copy(ob, ps)
                            drain_ct += 1
                            nc.sync.dma_start(out.ap()[:, qo, nsl], ob)

                    def emit_tail(t):
                        # denominator: one ones-matmul over the
                        # chain-accumulated exp sums, then normalize
                        qstart, w, h, ps_o, acc = t
                        ps_sum = ps2.tile([P, QT], fp32, tag="ps_sum")
                        nc.tensor.matmul(ps_sum[:, 0:w], ones, acc[:, 0:w],
                                         start=True, stop=True)
                        rec = ep.tile([P, QT], fp32, tag="rec")
                        nc.vector.reciprocal_approx_fast(rec[:, 0:w],
                                                         ps_sum[:, 0:w])
                        nc.vector.tensor_mul(
                            aoT_sb[:, h, qstart:qstart + w],
                            ps_o[:, 0:w], rec[:, 0:w])

                    # Each head's softmax tail (ones/reciprocal/normalize)
                    # and one deferred out-projection row block are emitted
                    # behind the NEXT head's first chunk, so the PE never
                    # waits on the DVE accumulate chain and the psum drains
                    # spread thinly over the ACT/DVE queues.
                    tail = None
                    pending = []  # deferred out-projection row blocks (qo)
                    for qstart, w in BLOCKS:
                        n_kt = (qstart + w) // P
                        for h in range(NH):
                            ps_o = ps2o.tile([P, QT], fp32, tag="ps_o")
                            acc = accp.tile([P, QT], bf16, tag="acc")
                            for m in range(n_kt):
                                # exact-causal trim: chunk m only feeds
                                # q columns >= m*P
                                lo = max(0, m * P - qstart)
                                ww = w - lo
                                ps_s = ps2s.tile([P, QT], fp32, tag="ps_s")
                                nc.tensor.matmul(
                                    ps_s[:, lo:lo + ww],
                                    kT_sb[:, h, m * P:(m + 1) * P],
                                    qT_sb[:, h, qstart + lo:qstart + w],
                                    start=True, stop=True)
                                tgt = acc if m == 0 else ax_pool.tile(
                                    [P, QT], bf16, tag="ax")
                                nc.scalar.activation(tgt[:, lo:lo + ww],
                                                     ps_s[:, lo:lo + ww],
                                                     Exp, scale=SCALE)
                                if m * P >= qstart:
                                    # diagonal chunk: triangle-mask the
                                    # first 128 columns it covers
                                    nc.vector.tensor_mul(
                                        tgt[:, lo:lo + P], tgt[:, lo:lo + P],
                                        mask_sb)
                                nc.tensor.matmul(
                                    ps_o[:, lo:lo + ww],
                                    v_sb[:, m, h * HD:(h + 1) * HD],
                                    tgt[:, lo:lo + ww],
                                    start=(m == 0), stop=(m == n_kt - 1),
                                    skip_group_check=True)
                                if m > 0:
                                    nc.vector.tensor_add(acc[:, lo:lo + ww],
                                                         acc[:, lo:lo + ww],
                                                         tgt[:, lo:lo + ww])
                                if m == 0:
                                    if tail is not None:
                                        emit_tail(tail)
                                        tail = None
                                    if pending:
                                        outproj_qo(pending.pop(0))
                            tail = (qstart, w, h, ps_o, acc)
                        pending.extend(range(qstart // P, (qstart + w) // P))
                    emit_tail(tail)
                    for qo in pending:
                        outproj_qo(qo)

    nc.compile()
    return nc


def _rope_tables():
    inv_freq = 1.0 / (ROPE_THETA ** (np.arange(0, HD, 2, dtype=np.float64) / HD))
    pos = np.arange(S, dtype=np.float64)
    freqs = np.outer(pos, inv_freq)                    # [S, HD/2]
    emb = np.concatenate([freqs, freqs], axis=-1)      # [S, HD]
    cos = np.cos(emb).T.astype(BF16)                   # [HD, S]
    sin = np.sin(emb).T.astype(np.float32)
    sin[: HD // 2] *= -1.0                             # fold rotate_half sign
    return cos, sin.astype(BF16)


def _pack_kd(a):
    """[D, N] -> [P, D//P, N] with d = ko*P + p."""
    d, n = a.shape
    return np.ascontiguousarray(
        a.reshape(d // P, P, n).transpose(1, 0, 2)).astype(BF16)


def make_in_maps(x, wq, wk, wv, wo):
    cosT, sinT = _rope_tables()
    i = np.arange(P)[:, None]
    j = np.arange(P)[None, :]
    mask = (i <= j).astype(BF16)                       # [128, 128] triangle

    in_maps = []
    for c in range(N_CORES):
        b, g = divmod(c, G)
        gsl = slice(g * DG, (g + 1) * DG)
        in_maps.append({
            "xT": _pack_kd(np.ascontiguousarray(x[b].T)),
            "wq": _pack_kd(wq[:, gsl]),
            "wk": _pack_kd(wk[:, gsl]),
            "wv": _pack_kd(wv[:, gsl]),
            "wo": _pack_kd(np.ascontiguousarray(wo[gsl, :])),
            "cosT": cosT,
            "sinT": sinT,
            "maskD": mask,
        })
    return in_maps


def assemble_output(results):
    """results: list of 8 dicts with 'out' [P, NKT, D] fp16."""
    full = np.empty((B, S, D), dtype=np.float32)
    for b in range(B):
        acc = None
        for g in range(G):
            r = results[b * G + g]["out"].astype(np.float32)
            part = r.transpose(1, 0, 2).reshape(S, D)
            acc = part if acc is None else acc + part
        full[b] = acc
    return full


def _get_module():
    global _BUILT
    if _BUILT is None:
        _BUILT = build_module()
    return _BUILT


def _install_trace_shim():
    """This image's antenv lacks axon_hooks; provide the NTFF profile hook
    via ctypes so trace=True (or BASS_TRACE=1) works instead of crashing,
    and skip the artifact bucket upload."""
    try:
        import antenv.axon_hooks  # noqa: F401
        return
    except ImportError:
        pass
    import types
    import ctypes
    import contextlib

    so_path = "/opt/axon/libaxon_pjrt.so"
    mod = types.ModuleType("antenv.axon_hooks")
    try:
        lib = ctypes.CDLL(so_path)
        lib.axon_start_nrt_profile.argtypes = [
            ctypes.POINTER(ctypes.c_int64), ctypes.c_size_t]
        lib.axon_start_nrt_profile.restype = ctypes.c_int64
        lib.axon_stop_nrt_profile.argtypes = [ctypes.c_char_p]
        lib.axon_stop_nrt_profile.restype = ctypes.c_int64

        @contextlib.contextmanager
        def _hook(output_dir, device_ids):
            import jax
            jax.devices()
            if device_ids:
                ids = (ctypes.c_int64 * len(device_ids))(*device_ids)
                rc = lib.axon_start_nrt_profile(ids, len(device_ids))
            else:
                rc = lib.axon_start_nrt_profile(None, 0)
            if rc != 0:
                raise RuntimeError(f"axon_start_nrt_profile rc={rc}")
            try:
                yield
            finally:
                lib.axon_stop_nrt_profile(str(output_dir).encode())

        mod.get_axon_ntff_profile_hook = lambda: _hook
    except OSError:
        mod.get_axon_ntff_profile_hook = lambda: None
    mod.set_axon_ntff_profile_hook = lambda h: None
    sys.modules["antenv.axon_hooks"] = mod

    from concourse import bass_utils
    bass_utils.upload_artifacts = lambda tmpdir: tmpdir


def run_on_hw(in_maps, trace=False, trace_cores=None):
    _install_trace_shim()
    from concourse import bass_utils
    nc = _get_module()
    return bass_utils.run_bass_kernel_spmd(
        nc, in_maps, core_ids=list(range(N_CORES)),
        trace=trace, trace_cores=trace_cores)


def kernel(x, wq, wk, wv, wo):
    x = np.asarray(x, dtype=np.float32)
    wq = np.asarray(wq, dtype=np.float32)
    wk = np.asarray(wk, dtype=np.float32)
    wv = np.asarray(wv, dtype=np.float32)
    wo = np.asarray(wo, dtype=np.float32)
    in_maps = make_in_maps(x, wq, wk, wv, wo)
    res = run_on_hw(in_maps, trace=False)
    return assemble_output(res.results)


# revision 16
# speedup vs baseline: 1.0012x; 1.0012x over previous
"""Llama attention layer (B=2, S=2048, D=2048, H=16, HD=128, RoPE, causal)
on 8 Trainium2 NeuronCores.

Sharding: core c -> (batch b = c//4, head group g = c%4 of 4 heads).
Each core computes q/k/v projections for its 512 columns of wq/wk/wv,
RoPE, causal attention for its 4 heads, and the out-projection against
its 512 rows of wo (a partial sum over head groups). The host sums the
4 partials per batch and stacks the 2 batches.

All device matmuls run in bf16 with fp32 PSUM accumulation (fp8 was
measured on-device: DoubleRow fp8 is exactly 2x bf16 MACs/cycle, but
plain fp8 quantization costs 3-5e-2 relative error — over the 2e-2
budget — and residual-corrected fp8 needs 3 half-rate GEMM terms =
1.5x bf16 time, so bf16 is optimal here). Softmax is computed without
max-subtraction (scores here are bounded ~|9|).

v2 changes over the first working version (352.8us -> ~329us):
- V-projection runs k-outer / m-inner in two passes of 8 PSUM groups so
  the first matmuls start as soon as x chunk 0 lands (DMA cascade of
  interleaved wv/x chunk transfers) instead of ~16us in.
- Exact-causal trimming: score/AV matmuls and the exp only cover
  q >= key-chunk-start column ranges (53% of the S x S square instead
  of the 62.5% block-causal coverage), with a single shared [128,128]
  triangle mask.
- Softmax denominator: exp chunks are chain-accumulated on DVE in bf16
  and reduced by ONE ones-matmul per (block, head) instead of one per
  chunk pair (saves ~33k PE cycles/core).
- Phase-1 PSUM is split 6+2 banks (ps1a/ps1b) with the last Q head-pair
  on ps1b, so the attention pools' allocation (pool-release WAR) waits
  only on ps1a — released when the last Q matmul retires — instead of
  on the final rope drains (+3.3us).
- Each head's softmax tail (ones/reciprocal/normalize) and one
  out-projection row block are deferred past the NEXT head's first
  chunk, so the PE never stalls on the DVE accumulate chain and psum
  drains spread thinly over the ACT/DVE queues instead of bursting.
- The final 512 q columns are processed as two 256-col blocks so the
  last out-projections have less attention serialized ahead of them.
- Output is written as fp16 partials (halves output DMA bytes; host
  sums partials in fp32).
"""

import os
import sys

import numpy as np
import ml_dtypes

if "/opt/trn_rl_repo" not in sys.path:
    sys.path.insert(0, "/opt/trn_rl_repo")

import concourse.bass as bass  # noqa: E402
import concourse.mybir as mybir  # noqa: E402
import concourse.bacc as bacc  # noqa: E402
import concourse.tile as tile  # noqa: E402

BF16 = ml_dtypes.bfloat16

B, S, D, H = 2, 2048, 2048, 16
HD = D // H            # 128, head dim
G = 4                  # head groups (cores per batch)
NH = H // G            # 4 heads per core
DG = NH * HD           # 512, per-core head width
P = 128
KO = D // P            # 16 k-subtiles over D
NKT = S // P           # 16 key chunks of 128
NQT = S // 512         # 4 q tiles of 512
QT = 512
ROPE_THETA = 10000.0
SCALE = 1.0 / float(np.sqrt(HD))

# q-column blocks for the attention phase: (start, width).
# The last 512 columns are split so the final out-projections have less
# attention work serialized in front of them (shorter kernel tail).
BLOCKS = [(0, 512), (512, 512), (1024, 512), (1536, 256), (1792, 256)]

N_CORES = 8

_BUILT = None  # (nc,) cache


def build_module():
    fp32 = mybir.dt.float32
    fp16 = mybir.dt.float16
    bf16 = mybir.dt.bfloat16

    nc = bacc.Bacc("TRN2", target_bir_lowering=False, debug=False,
                   num_devices=N_CORES, num_swdge_queues=4)

    xT = nc.dram_tensor("xT", [P, KO, S], bf16, kind="ExternalInput")
    wq = nc.dram_tensor("wq", [P, KO, DG], bf16, kind="ExternalInput")
    wk = nc.dram_tensor("wk", [P, KO, DG], bf16, kind="ExternalInput")
    wv = nc.dram_tensor("wv", [P, KO, DG], bf16, kind="ExternalInput")
    wo = nc.dram_tensor("wo", [P, NH, D], bf16, kind="ExternalInput")
    cosT = nc.dram_tensor("cosT", [P, S], bf16, kind="ExternalInput")
    sinT = nc.dram_tensor("sinT", [P, S], bf16, kind="ExternalInput")
    maskD = nc.dram_tensor("maskD", [P, P], bf16, kind="ExternalInput")
    out = nc.dram_tensor("out", [P, NKT, D], fp16, kind="ExternalOutput")

    Exp = mybir.ActivationFunctionType.Exp

    with tile.TileContext(nc) as tc:
        with tc.tile_pool(name="const", bufs=1) as const, \
             tc.tile_pool(name="big", bufs=1) as big:
            ones = const.tile([P, P], bf16)
            nc.vector.memset(ones, 1.0)
            mask_sb = const.tile([P, P], bf16)
            # dummy exp so the ACT Exp table loads during the DMA prefix,
            # not at the first real exp in the attention phase
            warm = const.tile([1, 1], fp32)
            nc.scalar.activation(warm, ones[0:1, 0:1],
                                 mybir.ActivationFunctionType.Exp)

            qT_sb = big.tile([P, NH, S], bf16)   # per head: [HD, S]
            kT_sb = big.tile([P, NH, S], bf16)
            v_sb = big.tile([P, NKT, DG], bf16)  # [key%128, keychunk, dg]
            aoT_sb = big.tile([P, NH, S], bf16)  # attention out^T

            # ---------------- phase 1: projections + RoPE ----------------
            # PSUM is split into a 6-bank pool (ps1a) and a 2-bank pool
            # (ps1b). Everything phase 1 does lands on ps1a except the V
            # groups m6/7/14/15 and the LAST Q head-pair, which go to ps1b.
            # The attention-phase pools for scores/ps_o/sum reuse only
            # ps1a's banks, so their allocation waits on ps1a's release
            # (done as the last Q matmuls retire) and NOT on the final
            # rope drains that hold ps1b — those only gate the out-proj
            # pool, which is first needed several microseconds later.
            with tc.tile_pool(name="w_pool", bufs=1) as w_pool, \
                 tc.tile_pool(name="rope", bufs=4) as rope, \
                 tc.tile_pool(name="ps1a", bufs=1, space="PSUM") as ps1a, \
                 tc.tile_pool(name="ps1b", bufs=1, space="PSUM") as ps1b:
                # DMA cascade: per-k interleaved wv/x chunks so the V loop
                # (k-outer) can start as soon as chunk 0 lands.
                wv_sb = w_pool.tile([P, KO, DG], bf16)
                xT_sb = w_pool.tile([P, KO, S], bf16)
                for k in range(KO):
                    nc.sync.dma_start(wv_sb[:, k, :], wv.ap()[:, k, :])
                    nc.sync.dma_start(xT_sb[:, k, :], xT.ap()[:, k, :])
                wk_sb = w_pool.tile([P, KO, DG], bf16)
                nc.sync.dma_start(wk_sb, wk.ap())
                wq_sb = w_pool.tile([P, KO, DG], bf16)
                nc.sync.dma_start(wq_sb, wq.ap())
                # cos/sin live in the rope pool (NOT w_pool): their last
                # readers are the final rope muls, and the attention-phase
                # SBUF pools fit inside w_pool's zone alone, so keeping
                # cos/sin out of w_pool lets those pools allocate as soon
                # as the last Q matmul (w_pool's true last reader) retires.
                cos_sb = rope.tile([P, S], bf16, tag="cos", bufs=1)
                nc.sync.dma_start(cos_sb, cosT.ap())
                sin_sb = rope.tile([P, S], bf16, tag="sin", bufs=1)
                nc.sync.dma_start(sin_sb, sinT.ap())
                nc.sync.dma_start(mask_sb, maskD.ap())

                def p1tile(last):
                    if last:
                        return ps1b.tile([P, QT], fp32, tag="psvb",
                                         name="psvb", bufs=2)
                    return ps1a.tile([P, QT], fp32, tag="psva",
                                     name="psva", bufs=6)

                # V: [keys, dg] natural layout. k-outer over two passes of
                # 8 m-groups (8 PSUM banks) so each arriving (wv, x) chunk
                # pair is consumed by 8 matmuls immediately.
                for half in range(2):
                    ms = list(range(8 * half, 8 * half + 8))
                    pss = {}
                    for m in ms:
                        pss[m] = p1tile(m % 8 >= 6)
                    for k in range(KO):
                        for m in ms:
                            nc.tensor.matmul(
                                pss[m], xT_sb[:, k, m * P:(m + 1) * P],
                                wv_sb[:, k, :],
                                start=(k == 0), stop=(k == KO - 1))
                    for m in ms:
                        nc.scalar.copy(v_sb[:, m, :], pss[m])

                # K then Q: [HD, S] transposed layout + RoPE.
                # Heads processed in pairs; psum groups rotate the 6-deep
                # ps1a ring so RoPE of one pair overlaps the next matmuls.
                for which, w_sb, dstT in (("k", wk_sb, kT_sb), ("q", wq_sb, qT_sb)):
                    for nt2 in range(2 * NQT):
                        nt, hp = divmod(nt2, 2)
                        sl = slice(nt * QT, (nt + 1) * QT)
                        heads = (2 * hp, 2 * hp + 1)
                        last_pair = which == "q" and nt2 == 2 * NQT - 1
                        pss = {}
                        for h in heads:
                            pss[h] = p1tile(last_pair)
                        for k in range(KO):
                            for h in heads:
                                nc.tensor.matmul(
                                    pss[h], w_sb[:, k, h * HD:(h + 1) * HD],
                                    xT_sb[:, k, sl],
                                    start=(k == 0), stop=(k == KO - 1))
                        for h in heads:
                            ps = pss[h]
                            dst = dstT[:, h, sl]
                            # rope: dst = ps * cos + swap(ps) * sin_signed.
                            # The swapped reads must come from PSUM (the SB-SB
                            # same-base-partition rule forbids them on SBUF);
                            # the straight read goes via a parallel ACT copy so
                            # the psum bank drains fast.
                            tmp = rope.tile([P, QT], bf16, tag="tmp")
                            nc.vector.tensor_mul(tmp[0:64], ps[64:128],
                                                 sin_sb[0:64, sl])
                            nc.vector.tensor_mul(tmp[64:128], ps[0:64],
                                                 sin_sb[64:128, sl])
                            qb = rope.tile([P, QT], bf16, tag="qb")
                            nc.scalar.copy(qb, ps)
                            nc.vector.tensor_mul(dst, qb, cos_sb[:, sl])
                            nc.vector.tensor_add(dst, dst, tmp)

            # ---------------- phases 2+3 ----------------
            with tc.tile_pool(name="big2", bufs=1) as big2:
                wo_sb = big2.tile([P, NH, D], bf16)
                nc.sync.dma_start(wo_sb, wo.ap())

                with tc.tile_pool(name="ax_pool", bufs=10) as ax_pool, \
                     tc.tile_pool(name="accp", bufs=3) as accp, \
                     tc.tile_pool(name="ep", bufs=3) as ep, \
                     tc.tile_pool(name="stage", bufs=6) as stage, \
                     tc.tile_pool(name="ps2s", bufs=3, space="PSUM") as ps2s, \
                     tc.tile_pool(name="ps2o", bufs=2, space="PSUM") as ps2o, \
                     tc.tile_pool(name="ps2", bufs=1, space="PSUM") as ps2, \
                     tc.tile_pool(name="ps3", bufs=2, space="PSUM") as ps3:

                    drain_ct = 0

                    def outproj_qo(qo):
                        nonlocal drain_ct
                        for n in range(D // QT):
                            nsl = slice(n * QT, (n + 1) * QT)
                            ps = ps3.tile([P, QT], fp32, tag="ps_out")
                            for h in range(NH):
                                nc.tensor.matmul(
                                    ps, aoT_sb[:, h, qo * P:(qo + 1) * P],
                                    wo_sb[:, h, nsl],
                                    start=(h == 0), stop=(h == NH - 1))
                            ob = stage.tile([P, QT], fp16, tag="ob")
                            # alternate the psum drains across ACT/DVE
                            if drain_ct % 2 == 0:
                                nc.scalar.copy(ob, ps)
                            else:
                                nc.vector.tensor_copy(ob, ps)
                            drain_ct += 1
                            nc.sync.dma_start(out.ap()[:, qo, nsl], ob)

                    def emit_tail(t):
                        # denominator: one ones-matmul over the
                        # chain-accumulated exp sums, then normalize
                        qstart, w, h, ps_o, acc = t
                        ps_sum = ps2.tile([P, QT], fp32, tag="ps_sum")
                        nc.tensor.matmul(ps_sum[:, 0:w], ones, acc[:, 0:w],
                                         start=True, stop=True)
                        rec = ep.tile([P, QT], fp32, tag="rec")
                        nc.vector.reciprocal_approx_fast(rec[:, 0:w],
                                                         ps_sum[:, 0:w])
                        nc.vector.tensor_mul(
                            aoT_sb[:, h, qstart:qstart + w],
                            ps_o[:, 0:w], rec[:, 0:w])

                    # Each head's softmax tail (ones/reciprocal/normalize)
                    # and one deferred out-projection row block are emitted
                    # behind the NEXT head's first chunk, so the PE never
                    # waits on the DVE accumulate chain and the psum drains
                    # spread thinly over the ACT/DVE queues.
                    tail = None
                    pending = []  # deferred out-projection row blocks (qo)
                    for qstart, w in BLOCKS:
                        n_kt = (qstart + w) // P
                        for h in range(NH):
                            ps_o = ps2o.tile([P, QT], fp32, tag="ps_o")
                            acc = accp.tile([P, QT], bf16, tag="acc")
                            for m in range(n_kt):
                                # exact-causal trim: chunk m only feeds
                                # q columns >= m*P
                                lo = max(0, m * P - qstart)
                                ww = w - lo
                                ps_s = ps2s.tile([P, QT], fp32, tag="ps_s")
                                nc.tensor.matmul(
                                    ps_s[:, lo:lo + ww],
                                    kT_sb[:, h, m * P:(m + 1) * P],
                                    qT_sb[:, h, qstart + lo:qstart + w],
                                    start=True, stop=True)
                                tgt = acc if m == 0 else ax_pool.tile(
                                    [P, QT], bf16, tag="ax")
                                nc.scalar.activation(tgt[:, lo:lo + ww],
                                                     ps_s[:, lo:lo + ww],
                                                     Exp, scale=SCALE)
                                if m * P >= qstart:
                                    # diagonal chunk: triangle-mask the
                                    # first 128 columns it covers
                                    nc.vector.tensor_mul(
                                        tgt[:, lo:lo + P], tgt[:, lo:lo + P],
                                        mask_sb)
                                nc.tensor.matmul(
                                    ps_o[:, lo:lo + ww],
                                    v_sb[:, m, h * HD:(h + 1) * HD],
                                    tgt[:, lo:lo + ww],
                                    start=(m == 0), stop=(m == n_kt - 1),
                                    skip_group_check=True)
                                if m > 0:
                                    nc.vector.tensor_add(acc[:, lo:lo + ww],
                                                         acc[:, lo:lo + ww],
                                                         tgt[:, lo:lo + ww])
                                if m == 0:
                                    if tail is not None:
                                        emit_tail(tail)
                                        tail = None
                                    if pending:
                                        outproj_qo(pending.pop(0))
                            tail = (qstart, w, h, ps_o, acc)
                        pending.extend(range(qstart // P, (qstart + w) // P))
                    emit_tail(tail)
                    for qo in pending:
                        outproj_qo(qo)

    nc.compile()
    return nc


def _rope_tables():
    inv_freq = 1.0 / (ROPE_THETA ** (np.arange(0, HD, 2, dtype=np.float64) / HD))
    pos = np.arange(S, dtype=np.float64)
    freqs = np.outer(pos, inv_freq)                    # [S, HD/2]
    emb = np.concatenate([freqs, freqs], axis=-1)      # [S, HD]
    cos = np.cos(emb).T.astype(BF16)                   # [HD, S]
    sin = np.sin(emb).T.astype(np.float32)
    sin[: HD // 2] *= -1.0                             # fold rotate_half sign
    return cos, sin.astype(BF16)


def _pack_kd(a):
    """[D, N] -> [P, D//P, N] with d = ko*P + p."""
    d, n = a.shape
    return np.ascontiguousarray(
        a.reshape(d // P, P, n).transpose(1, 0, 2)).astype(BF16)


def make_in_maps(x, wq, wk, wv, wo):
    cosT, sinT = _rope_tables()
    i = np.arange(P)[:, None]
    j = np.arange(P)[None, :]
    mask = (i <= j).astype(BF16)                       # [128, 128] triangle

    in_maps = []
    for c in range(N_CORES):
        b, g = divmod(c, G)
        gsl = slice(g * DG, (g + 1) * DG)
        in_maps.append({
            "xT": _pack_kd(np.ascontiguousarray(x[b].T)),
            "wq": _pack_kd(wq[:, gsl]),
            "wk": _pack_kd(wk[:, gsl]),
            "wv": _pack_kd(wv[:, gsl]),
            "wo": _pack_kd(np.ascontiguousarray(wo[gsl, :])),
            "cosT": cosT,
            "sinT": sinT,
            "maskD": mask,
        })
    return in_maps


def assemble_output(results):
    """results: list of 8 dicts with 'out' [P, NKT, D] fp16."""
    full = np.empty((B, S, D), dtype=np.float32)
    for b in range(B):
        acc = None
        for g in range(G):
            r = results[b * G + g]["out"].astype(np.float32)
            part = r.transpose(1, 0, 2).reshape(S, D)
            acc = part if acc is None else acc + part
        full[b] = acc
    return full


def _get_module():
    global _BUILT
    if _BUILT is None:
        _BUILT = build_module()
    return _BUILT


def _install_trace_shim():
    """This image's antenv lacks axon_hooks; provide the NTFF profile hook
    via ctypes so trace=True (or BASS_TRACE=1) works instead of crashing,
    and skip the artifact bucket upload."""
    try:
        import antenv.axon_hooks  # noqa: F401
        return
    except ImportError:
        pass
    import types
    import ctypes
    import contextlib

    so_path = "/opt/axon/libaxon_pjrt.so"
    mod = types.ModuleType("antenv.axon_hooks")
    try:
        lib = ctypes.CDLL(so_path)
        lib.axon_start_nrt_profile.argtypes = [
            ctypes.POINTER(ctypes.c_int64), ctypes.c_size_t]
        lib.axon_start_nrt_profile.restype = ctypes.c_int64
        lib.axon_stop_nrt_profile.argtypes = [ctypes.c_char_p]
        lib.axon_stop_nrt_profile.restype = ctypes.c_int64

        @contextlib.contextmanager
        def _hook(output_dir, device_ids):
            import jax
            jax.devices()
            if device_ids:
                ids = (ctypes.c_int64 * len(device_ids))(*device_ids)
                rc = lib.axon_start_nrt_profile(ids, len(device_ids))
            else:
                rc = lib.axon_start_nrt_profile(None, 0)
            if rc != 0:
                raise RuntimeError(f"axon_start_nrt_profile rc={rc}")
            try:
                yield
            finally:
                lib.axon_stop_nrt_profile(str(output_dir).encode())

        mod.get_axon_ntff_profile_hook = lambda: _hook
    except OSError:
        mod.get_axon_ntff_profile_hook = lambda: None
    mod.set_axon_ntff_profile_hook = lambda h: None
    sys.modules["antenv.axon_hooks"] = mod

    from concourse import bass_utils
    bass_utils.upload_artifacts = lambda tmpdir: tmpdir


def run_on_hw(in_maps, trace=False, trace_cores=None):
    _install_trace_shim()
    from concourse import bass_utils
    nc = _get_module()
    return bass_utils.run_bass_kernel_spmd(
        nc, in_maps, core_ids=list(range(N_CORES)),
        trace=trace, trace_cores=trace_cores)


def kernel(x, wq, wk, wv, wo):
    x = np.asarray(x, dtype=np.float32)
    wq = np.asarray(wq, dtype=np.float32)
    wk = np.asarray(wk, dtype=np.float32)
    wv = np.asarray(wv, dtype=np.float32)
    wo = np.asarray(wo, dtype=np.float32)
    in_maps = make_in_maps(x, wq, wk, wv, wo)
    res = run_on_hw(in_maps, trace=False)
    return assemble_output(res.results)


# revision 18
# speedup vs baseline: 1.0029x; 1.0016x over previous
"""Llama attention layer (B=2, S=2048, D=2048, H=16, HD=128, RoPE, causal)
on 8 Trainium2 NeuronCores.

Sharding: core c -> (batch b = c//4, head group g = c%4 of 4 heads).
Each core computes q/k/v projections for its 512 columns of wq/wk/wv,
RoPE, causal attention for its 4 heads, and the out-projection against
its 512 rows of wo (a partial sum over head groups). The host sums the
4 partials per batch and stacks the 2 batches.

All device matmuls run in bf16 with fp32 PSUM accumulation (fp8 was
measured on-device: DoubleRow fp8 is exactly 2x bf16 MACs/cycle, but
plain fp8 quantization costs 3-5e-2 relative error — over the 2e-2
budget — and residual-corrected fp8 needs 3 half-rate GEMM terms =
1.5x bf16 time, so bf16 is optimal here). Softmax is computed without
max-subtraction (scores here are bounded ~|9|).

v2 changes over the first working version (352.8us -> ~329us):
- V-projection runs k-outer / m-inner in two passes of 8 PSUM groups so
  the first matmuls start as soon as x chunk 0 lands (DMA cascade of
  interleaved wv/x chunk transfers) instead of ~16us in.
- Exact-causal trimming: score/AV matmuls and the exp only cover
  q >= key-chunk-start column ranges (53% of the S x S square instead
  of the 62.5% block-causal coverage), with a single shared [128,128]
  triangle mask.
- Softmax denominator: exp chunks are chain-accumulated on DVE in bf16
  and reduced by ONE ones-matmul per (block, head) instead of one per
  chunk pair (saves ~33k PE cycles/core).
- Phase-1 PSUM is split 6+2 banks (ps1a/ps1b) with the last Q head-pair
  on ps1b, so the attention pools' allocation (pool-release WAR) waits
  only on ps1a — released when the last Q matmul retires — instead of
  on the final rope drains (+3.3us).
- Each head's softmax tail (ones/reciprocal/normalize) and one
  out-projection row block are deferred past the NEXT head's first
  chunk, so the PE never stalls on the DVE accumulate chain and psum
  drains spread thinly over the ACT/DVE queues instead of bursting.
- The final 512 q columns are processed as two 256-col blocks so the
  last out-projections have less attention serialized ahead of them.
- Output is written as fp16 partials (halves output DMA bytes; host
  sums partials in fp32).
"""

import os
import sys

import numpy as np
import ml_dtypes

if "/opt/trn_rl_repo" not in sys.path:
    sys.path.insert(0, "/opt/trn_rl_repo")

import concourse.bass as bass  # noqa: E402
import concourse.mybir as mybir  # noqa: E402
import concourse.bacc as bacc  # noqa: E402
import concourse.tile as tile  # noqa: E402

BF16 = ml_dtypes.bfloat16

B, S, D, H = 2, 2048, 2048, 16
HD = D // H            # 128, head dim
G = 4                  # head groups (cores per batch)
NH = H // G            # 4 heads per core
DG = NH * HD           # 512, per-core head width
P = 128
KO = D // P            # 16 k-subtiles over D
NKT = S // P           # 16 key chunks of 128
NQT = S // 512         # 4 q tiles of 512
QT = 512
ROPE_THETA = 10000.0
SCALE = 1.0 / float(np.sqrt(HD))

# q-column blocks for the attention phase: (start, width).
# The last 512 columns are split so the final out-projections have less
# attention work serialized in front of them (shorter kernel tail).
BLOCKS = [(0, 512), (512, 512), (1024, 512), (1536, 256), (1792, 256)]

N_CORES = 8

_BUILT = None  # (nc,) cache


def build_module():
    fp32 = mybir.dt.float32
    fp16 = mybir.dt.float16
    bf16 = mybir.dt.bfloat16

    nc = bacc.Bacc("TRN2", target_bir_lowering=False, debug=False,
                   num_devices=N_CORES, num_swdge_queues=4)

    xT = nc.dram_tensor("xT", [P, KO, S], bf16, kind="ExternalInput")
    wq = nc.dram_tensor("wq", [P, KO, DG], bf16, kind="ExternalInput")
    wk = nc.dram_tensor("wk", [P, KO, DG], bf16, kind="ExternalInput")
    wv = nc.dram_tensor("wv", [P, KO, DG], bf16, kind="ExternalInput")
    wo = nc.dram_tensor("wo", [P, NH, D], bf16, kind="ExternalInput")
    cosT = nc.dram_tensor("cosT", [P, S], bf16, kind="ExternalInput")
    sinT = nc.dram_tensor("sinT", [P, S], bf16, kind="ExternalInput")
    maskD = nc.dram_tensor("maskD", [P, P], bf16, kind="ExternalInput")
    out = nc.dram_tensor("out", [P, NKT, D], fp16, kind="ExternalOutput")

    Exp = mybir.ActivationFunctionType.Exp

    with tile.TileContext(nc) as tc:
        with tc.tile_pool(name="const", bufs=1) as const, \
             tc.tile_pool(name="big", bufs=1) as big:
            ones = const.tile([P, P], bf16)
            nc.vector.memset(ones, 1.0)
            mask_sb = const.tile([P, P], bf16)
            # dummy exp so the ACT Exp table loads during the DMA prefix,
            # not at the first real exp in the attention phase
            warm = const.tile([1, 1], fp32)
            nc.scalar.activation(warm, ones[0:1, 0:1],
                                 mybir.ActivationFunctionType.Exp)

            qT_sb = big.tile([P, NH, S], bf16)   # per head: [HD, S]
            kT_sb = big.tile([P, NH, S], bf16)
            v_sb = big.tile([P, NKT, DG], bf16)  # [key%128, keychunk, dg]
            aoT_sb = big.tile([P, NH, S], bf16)  # attention out^T

            # ---------------- phase 1: projections + RoPE ----------------
            # PSUM is split into a 6-bank pool (ps1a) and a 2-bank pool
            # (ps1b). Everything phase 1 does lands on ps1a except the V
            # groups m6/7/14/15 and the LAST Q head-pair, which go to ps1b.
            # The attention-phase pools for scores/ps_o/sum reuse only
            # ps1a's banks, so their allocation waits on ps1a's release
            # (done as the last Q matmuls retire) and NOT on the final
            # rope drains that hold ps1b — those only gate the out-proj
            # pool, which is first needed several microseconds later.
            with tc.tile_pool(name="w_pool", bufs=1) as w_pool, \
                 tc.tile_pool(name="rope", bufs=4) as rope, \
                 tc.tile_pool(name="ps1a", bufs=1, space="PSUM") as ps1a, \
                 tc.tile_pool(name="ps1b", bufs=1, space="PSUM") as ps1b:
                # DMA cascade: per-k interleaved wv/x chunks so the V loop
                # (k-outer) can start as soon as chunk 0 lands.
                wv_sb = w_pool.tile([P, KO, DG], bf16)
                xT_sb = w_pool.tile([P, KO, S], bf16)
                for k in range(KO):
                    nc.sync.dma_start(wv_sb[:, k, :], wv.ap()[:, k, :])
                    nc.sync.dma_start(xT_sb[:, k, :], xT.ap()[:, k, :])
                wk_sb = w_pool.tile([P, KO, DG], bf16)
                nc.sync.dma_start(wk_sb, wk.ap())
                wq_sb = w_pool.tile([P, KO, DG], bf16)
                nc.sync.dma_start(wq_sb, wq.ap())
                # cos/sin live in the rope pool (NOT w_pool): their last
                # readers are the final rope muls, and the attention-phase
                # SBUF pools fit inside w_pool's zone alone, so keeping
                # cos/sin out of w_pool lets those pools allocate as soon
                # as the last Q matmul (w_pool's true last reader) retires.
                cos_sb = rope.tile([P, S], bf16, tag="cos", bufs=1)
                nc.sync.dma_start(cos_sb, cosT.ap())
                sin_sb = rope.tile([P, S], bf16, tag="sin", bufs=1)
                nc.sync.dma_start(sin_sb, sinT.ap())
                nc.sync.dma_start(mask_sb, maskD.ap())

                def p1tile(last):
                    if last:
                        return ps1b.tile([P, QT], fp32, tag="psvb",
                                         name="psvb", bufs=2)
                    return ps1a.tile([P, QT], fp32, tag="psva",
                                     name="psva", bufs=6)

                # V: [keys, dg] natural layout. k-outer over two passes of
                # 8 m-groups (8 PSUM banks) so each arriving (wv, x) chunk
                # pair is consumed by 8 matmuls immediately.
                for half in range(2):
                    ms = list(range(8 * half, 8 * half + 8))
                    pss = {}
                    for m in ms:
                        pss[m] = p1tile(m % 8 >= 6)
                    for k in range(KO):
                        for m in ms:
                            nc.tensor.matmul(
                                pss[m], xT_sb[:, k, m * P:(m + 1) * P],
                                wv_sb[:, k, :],
                                start=(k == 0), stop=(k == KO - 1))
                    for m in ms:
                        nc.scalar.copy(v_sb[:, m, :], pss[m])

                # K then Q: [HD, S] transposed layout + RoPE.
                # Heads processed in pairs; psum groups rotate the 6-deep
                # ps1a ring so RoPE of one pair overlaps the next matmuls.
                for which, w_sb, dstT in (("k", wk_sb, kT_sb), ("q", wq_sb, qT_sb)):
                    for nt2 in range(2 * NQT):
                        nt, hp = divmod(nt2, 2)
                        sl = slice(nt * QT, (nt + 1) * QT)
                        heads = (2 * hp, 2 * hp + 1)
                        last_pair = which == "q" and nt2 == 2 * NQT - 1
                        pss = {}
                        for h in heads:
                            pss[h] = p1tile(last_pair)
                        for k in range(KO):
                            for h in heads:
                                nc.tensor.matmul(
                                    pss[h], w_sb[:, k, h * HD:(h + 1) * HD],
                                    xT_sb[:, k, sl],
                                    start=(k == 0), stop=(k == KO - 1))
                        for h in heads:
                            ps = pss[h]
                            dst = dstT[:, h, sl]
                            # rope: dst = ps * cos + swap(ps) * sin_signed.
                            # The swapped reads must come from PSUM (the SB-SB
                            # same-base-partition rule forbids them on SBUF);
                            # the straight read goes via a parallel ACT copy so
                            # the psum bank drains fast.
                            tmp = rope.tile([P, QT], bf16, tag="tmp")
                            nc.vector.tensor_mul(tmp[0:64], ps[64:128],
                                                 sin_sb[0:64, sl])
                            nc.vector.tensor_mul(tmp[64:128], ps[0:64],
                                                 sin_sb[64:128, sl])
                            qb = rope.tile([P, QT], bf16, tag="qb")
                            nc.scalar.copy(qb, ps)
                            nc.vector.tensor_mul(dst, qb, cos_sb[:, sl])
                            nc.vector.tensor_add(dst, dst, tmp)

            # ---------------- phases 2+3 ----------------
            with tc.tile_pool(name="big2", bufs=1) as big2:
                wo_sb = big2.tile([P, NH, D], bf16)
                nc.sync.dma_start(wo_sb, wo.ap())

                with tc.tile_pool(name="ax_pool", bufs=10) as ax_pool, \
                     tc.tile_pool(name="accp", bufs=3) as accp, \
                     tc.tile_pool(name="ep", bufs=3) as ep, \
                     tc.tile_pool(name="stage", bufs=6) as stage, \
                     tc.tile_pool(name="ps2s", bufs=3, space="PSUM") as ps2s, \
                     tc.tile_pool(name="ps2o", bufs=2, space="PSUM") as ps2o, \
                     tc.tile_pool(name="ps2", bufs=1, space="PSUM") as ps2, \
                     tc.tile_pool(name="ps3", bufs=2, space="PSUM") as ps3:

                    drain_ct = 0

                    def outproj_qo(qo):
                        nonlocal drain_ct
                        for n in range(D // QT):
                            nsl = slice(n * QT, (n + 1) * QT)
                            ps = ps3.tile([P, QT], fp32, tag="ps_out")
                            for h in range(NH):
                                nc.tensor.matmul(
                                    ps, aoT_sb[:, h, qo * P:(qo + 1) * P],
                                    wo_sb[:, h, nsl],
                                    start=(h == 0), stop=(h == NH - 1))
                            ob = stage.tile([P, QT], fp16, tag="ob")
                            # alternate the psum drains across ACT/DVE
                            if drain_ct % 2 == 0:
                                nc.scalar.copy(ob, ps)
                            else:
                                nc.vector.tensor_copy(ob, ps)
                            drain_ct += 1
                            nc.sync.dma_start(out.ap()[:, qo, nsl], ob)

                    def emit_tail(t):
                        # denominator: one ones-matmul over the
                        # chain-accumulated exp sums, then normalize
                        qstart, w, h, ps_o, acc = t
                        ps_sum = ps2.tile([P, QT], fp32, tag="ps_sum")
                        nc.tensor.matmul(ps_sum[:, 0:w], ones, acc[:, 0:w],
                                         start=True, stop=True)
                        rec = ep.tile([P, QT], fp32, tag="rec")
                        nc.vector.reciprocal_approx_fast(rec[:, 0:w],
                                                         ps_sum[:, 0:w])
                        nc.vector.tensor_mul(
                            aoT_sb[:, h, qstart:qstart + w],
                            ps_o[:, 0:w], rec[:, 0:w])

                    # Each head's softmax tail (ones/reciprocal/normalize)
                    # and one deferred out-projection row block are emitted
                    # behind the NEXT head's first chunk, so the PE never
                    # waits on the DVE accumulate chain and the psum drains
                    # spread thinly over the ACT/DVE queues.
                    tail = None
                    pending = []  # deferred out-projection row blocks (qo)
                    for qstart, w in BLOCKS:
                        n_kt = (qstart + w) // P
                        for h in range(NH):
                            ps_o = ps2o.tile([P, QT], fp32, tag="ps_o")
                            acc = accp.tile([P, QT], bf16, tag="acc")
                            for m in range(n_kt):
                                # exact-causal trim: chunk m only feeds
                                # q columns >= m*P
                                lo = max(0, m * P - qstart)
                                ww = w - lo
                                ps_s = ps2s.tile([P, QT], fp32, tag="ps_s")
                                nc.tensor.matmul(
                                    ps_s[:, lo:lo + ww],
                                    kT_sb[:, h, m * P:(m + 1) * P],
                                    qT_sb[:, h, qstart + lo:qstart + w],
                                    start=True, stop=True)
                                tgt = acc if m == 0 else ax_pool.tile(
                                    [P, QT], bf16, tag="ax")
                                nc.scalar.activation(tgt[:, lo:lo + ww],
                                                     ps_s[:, lo:lo + ww],
                                                     Exp, scale=SCALE)
                                if m * P >= qstart:
                                    # diagonal chunk: triangle-mask the
                                    # first 128 columns it covers
                                    nc.vector.tensor_mul(
                                        tgt[:, lo:lo + P], tgt[:, lo:lo + P],
                                        mask_sb)
                                nc.tensor.matmul(
                                    ps_o[:, lo:lo + ww],
                                    v_sb[:, m, h * HD:(h + 1) * HD],
                                    tgt[:, lo:lo + ww],
                                    start=(m == 0), stop=(m == n_kt - 1),
                                    skip_group_check=True)
                                if m > 0:
                                    nc.vector.tensor_add(acc[:, lo:lo + ww],
                                                         acc[:, lo:lo + ww],
                                                         tgt[:, lo:lo + ww])
                                if m == 0:
                                    if tail is not None:
                                        emit_tail(tail)
                                        tail = None
                                    if pending:
                                        outproj_qo(pending.pop(0))
                            tail = (qstart, w, h, ps_o, acc)
                        pending.extend(range(qstart // P, (qstart + w) // P))
                    emit_tail(tail)
                    for qo in pending:
                        outproj_qo(qo)

    nc.compile()
    return nc


def _rope_tables():
    inv_freq = 1.0 / (ROPE_THETA ** (np.arange(0, HD, 2, dtype=np.float64) / HD))
    pos = np.arange(S, dtype=np.float64)
    freqs = np.outer(pos, inv_freq)                    # [S, HD/2]
    emb = np.concatenate([freqs, freqs], axis=-1)      # [S, HD]
    cos = np.cos(emb).T.astype(BF16)                   # [HD, S]
    sin = np.sin(emb).T.astype(np.float32)
    sin[: HD // 2] *= -1.0                             # fold rotate_half sign
    return cos, sin.astype(BF16)


def _pack_kd(a):
    """[D, N] -> [P, D//P, N] with d = ko*P + p."""
    d, n = a.shape
    return np.ascontiguousarray(
        a.reshape(d // P, P, n).transpose(1, 0, 2)).astype(BF16)


def make_in_maps(x, wq, wk, wv, wo):
    cosT, sinT = _rope_tables()
    i = np.arange(P)[:, None]
    j = np.arange(P)[None, :]
    mask = (i <= j).astype(BF16)                       # [128, 128] triangle

    in_maps = []
    for c in range(N_CORES):
        b, g = divmod(c, G)
        gsl = slice(g * DG, (g + 1) * DG)
        in_maps.append({
            "xT": _pack_kd(np.ascontiguousarray(x[b].T)),
            "wq": _pack_kd(wq[:, gsl]),
            "wk": _pack_kd(wk[:, gsl]),
            "wv": _pack_kd(wv[:, gsl]),
            "wo": _pack_kd(np.ascontiguousarray(wo[gsl, :])),
            "cosT": cosT,
            "sinT": sinT,
            "maskD": mask,
        })
    return in_maps


def assemble_output(results):
    """results: list of 8 dicts with 'out' [P, NKT, D] fp16."""
    full = np.empty((B, S, D), dtype=np.float32)
    for b in range(B):
        acc = None
        for g in range(G):
            r = results[b * G + g]["out"].astype(np.float32)
            part = r.transpose(1, 0, 2).reshape(S, D)
            acc = part if acc is None else acc + part
        full[b] = acc
    return full


def _get_module():
    global _BUILT
    if _BUILT is None:
        _BUILT = build_module()
    return _BUILT


def _install_trace_shim():
    """This image's antenv lacks axon_hooks; provide the NTFF profile hook
    via ctypes so trace=True (or BASS_TRACE=1) works instead of crashing,
    and skip the artifact bucket upload."""
    try:
        import antenv.axon_hooks  # noqa: F401
        return
    except ImportError:
        pass
    import types
    import ctypes
    import contextlib

    so_path = "/opt/axon/libaxon_pjrt.so"
    mod = types.ModuleType("antenv.axon_hooks")
    try:
        lib = ctypes.CDLL(so_path)
        lib.axon_start_nrt_profile.argtypes = [
            ctypes.POINTER(ctypes.c_int64), ctypes.c_size_t]
        lib.axon_start_nrt_profile.restype = ctypes.c_int64
        lib.axon_stop_nrt_profile.argtypes = [ctypes.c_char_p]
        lib.axon_stop_nrt_profile.restype = ctypes.c_int64

        @contextlib.contextmanager
        def _hook(output_dir, device_ids):
            import jax
            jax.devices()
            if device_ids:
                ids = (ctypes.c_int64 * len(device_ids))(*device_ids)
                rc = lib.axon_start_nrt_profile(ids, len(device_ids))
            else:
                rc = lib.axon_start_nrt_profile(None, 0)
            if rc != 0:
                raise RuntimeError(f"axon_start_nrt_profile rc={rc}")
            try:
                yield
            finally:
                lib.axon_stop_nrt_profile(str(output_dir).encode())

        mod.get_axon_ntff_profile_hook = lambda: _hook
    except OSError:
        mod.get_axon_ntff_profile_hook = lambda: None
    mod.set_axon_ntff_profile_hook = lambda h: None
    sys.modules["antenv.axon_hooks"] = mod

    from concourse import bass_utils
    bass_utils.upload_artifacts = lambda tmpdir: tmpdir


def run_on_hw(in_maps, trace=False, trace_cores=None):
    _install_trace_shim()
    from concourse import bass_utils
    nc = _get_module()
    return bass_utils.run_bass_kernel_spmd(
        nc, in_maps, core_ids=list(range(N_CORES)),
        trace=trace, trace_cores=trace_cores)


def kernel(x, wq, wk, wv, wo):
    x = np.asarray(x, dtype=np.float32)
    wq = np.asarray(wq, dtype=np.float32)
    wk = np.asarray(wk, dtype=np.float32)
    wv = np.asarray(wv, dtype=np.float32)
    wo = np.asarray(wo, dtype=np.float32)
    in_maps = make_in_maps(x, wq, wk, wv, wo)
    res = run_on_hw(in_maps, trace=False)
    return assemble_output(res.results)


# revision 19
# speedup vs baseline: 1.0145x; 1.0116x over previous
"""Llama attention layer (B=2, S=2048, D=2048, H=16, HD=128, RoPE, causal)
on 8 Trainium2 NeuronCores.

Sharding: core c -> (batch b = c//4, head group g = c%4 of 4 heads).
Each core computes q/k/v projections for its 512 columns of wq/wk/wv,
RoPE, causal attention for its 4 heads, and the out-projection against
its 512 rows of wo (a partial sum over head groups). The host sums the
4 partials per batch and stacks the 2 batches.

All device matmuls run in bf16 with fp32 PSUM accumulation (fp8 was
measured on-device: DoubleRow fp8 is exactly 2x bf16 MACs/cycle, but
plain fp8 quantization costs 3-5e-2 relative error — over the 2e-2
budget — and residual-corrected fp8 needs 3 half-rate GEMM terms =
1.5x bf16 time, so bf16 is optimal here). Softmax is computed without
max-subtraction (scores here are bounded ~|9|).

v2 changes over the first working version (352.8us -> ~329us):
- V-projection runs k-outer / m-inner in two passes of 8 PSUM groups so
  the first matmuls start as soon as x chunk 0 lands (DMA cascade of
  interleaved wv/x chunk transfers) instead of ~16us in.
- Exact-causal trimming: score/AV matmuls and the exp only cover
  q >= key-chunk-start column ranges (53% of the S x S square instead
  of the 62.5% block-causal coverage), with a single shared [128,128]
  triangle mask.
- Softmax denominator: exp chunks are chain-accumulated on DVE in bf16
  and reduced by ONE ones-matmul per (block, head) instead of one per
  chunk pair (saves ~33k PE cycles/core).
- Phase-1 PSUM is split 6+2 banks (ps1a/ps1b) with the last Q head-pair
  on ps1b, so the attention pools' allocation (pool-release WAR) waits
  only on ps1a — released when the last Q matmul retires — instead of
  on the final rope drains (+3.3us).
- Each head's softmax tail (ones/reciprocal/normalize) and one
  out-projection row block are deferred past the NEXT head's first
  chunk, so the PE never stalls on the DVE accumulate chain and psum
  drains spread thinly over the ACT/DVE queues instead of bursting.
- The final 512 q columns are processed as two 256-col blocks so the
  last out-projections have less attention serialized ahead of them.
- Output is written as fp16 partials (halves output DMA bytes; host
  sums partials in fp32).
"""

import os
import sys

import numpy as np
import ml_dtypes

if "/opt/trn_rl_repo" not in sys.path:
    sys.path.insert(0, "/opt/trn_rl_repo")

import concourse.bass as bass  # noqa: E402
import concourse.mybir as mybir  # noqa: E402
import concourse.bacc as bacc  # noqa: E402
import concourse.tile as tile  # noqa: E402

BF16 = ml_dtypes.bfloat16

B, S, D, H = 2, 2048, 2048, 16
HD = D // H            # 128, head dim
G = 4                  # head groups (cores per batch)
NH = H // G            # 4 heads per core
DG = NH * HD           # 512, per-core head width
P = 128
KO = D // P            # 16 k-subtiles over D
NKT = S // P           # 16 key chunks of 128
NQT = S // 512         # 4 q tiles of 512
QT = 512
ROPE_THETA = 10000.0
SCALE = 1.0 / float(np.sqrt(HD))

# q-column blocks for the attention phase: (start, width).
# The last 512 columns are split so the final out-projections have less
# attention work serialized in front of them (shorter kernel tail).
BLOCKS = [(0, 512), (512, 512), (1024, 512), (1536, 256), (1792, 256)]

N_CORES = 8

_BUILT = None  # (nc,) cache


def build_module():
    fp32 = mybir.dt.float32
    fp16 = mybir.dt.float16
    bf16 = mybir.dt.bfloat16

    nc = bacc.Bacc("TRN2", target_bir_lowering=False, debug=False,
                   num_devices=N_CORES, num_swdge_queues=4)

    xT = nc.dram_tensor("xT", [P, KO, S], bf16, kind="ExternalInput")
    wq = nc.dram_tensor("wq", [P, KO, DG], bf16, kind="ExternalInput")
    wk = nc.dram_tensor("wk", [P, KO, DG], bf16, kind="ExternalInput")
    wv = nc.dram_tensor("wv", [P, KO, DG], bf16, kind="ExternalInput")
    wo = nc.dram_tensor("wo", [P, NH, D], bf16, kind="ExternalInput")
    cosT = nc.dram_tensor("cosT", [P, S], bf16, kind="ExternalInput")
    sinT = nc.dram_tensor("sinT", [P, S], bf16, kind="ExternalInput")
    maskD = nc.dram_tensor("maskD", [P, P], bf16, kind="ExternalInput")
    out = nc.dram_tensor("out", [P, NKT, D], fp16, kind="ExternalOutput")

    Exp = mybir.ActivationFunctionType.Exp

    with tile.TileContext(nc) as tc:
        with tc.tile_pool(name="const", bufs=1) as const, \
             tc.tile_pool(name="big", bufs=1) as big:
            ones = const.tile([P, P], bf16)
            nc.vector.memset(ones, 1.0)
            mask_sb = const.tile([P, P], bf16)
            # dummy exp so the ACT Exp table loads during the DMA prefix,
            # not at the first real exp in the attention phase
            warm = const.tile([1, 1], fp32)
            nc.scalar.activation(warm, ones[0:1, 0:1],
                                 mybir.ActivationFunctionType.Exp)

            qT_sb = big.tile([P, NH, S], bf16)   # per head: [HD, S]
            kT_sb = big.tile([P, NH, S], bf16)
            v_sb = big.tile([P, NKT, DG], bf16)  # [key%128, keychunk, dg]
            aoT_sb = big.tile([P, NH, S], bf16)  # attention out^T

            # ---------------- phase 1: projections + RoPE ----------------
            # PSUM is split into a 6-bank pool (ps1a) and a 2-bank pool
            # (ps1b). Everything phase 1 does lands on ps1a except the V
            # groups m6/7/14/15 and the LAST Q head-pair, which go to ps1b.
            # The attention-phase pools for scores/ps_o/sum reuse only
            # ps1a's banks, so their allocation waits on ps1a's release
            # (done as the last Q matmuls retire) and NOT on the final
            # rope drains that hold ps1b — those only gate the out-proj
            # pool, which is first needed several microseconds later.
            with tc.tile_pool(name="w_pool", bufs=1) as w_pool, \
                 tc.tile_pool(name="rope", bufs=4) as rope, \
                 tc.tile_pool(name="ps1a", bufs=1, space="PSUM") as ps1a, \
                 tc.tile_pool(name="ps1b", bufs=1, space="PSUM") as ps1b:
                # DMA cascade: per-k interleaved wv/x chunks so the V loop
                # (k-outer) can start as soon as chunk 0 lands.
                wv_sb = w_pool.tile([P, KO, DG], bf16)
                xT_sb = w_pool.tile([P, KO, S], bf16)
                for k in range(KO):
                    nc.sync.dma_start(wv_sb[:, k, :], wv.ap()[:, k, :])
                    nc.sync.dma_start(xT_sb[:, k, :], xT.ap()[:, k, :])
                wk_sb = w_pool.tile([P, KO, DG], bf16)
                nc.sync.dma_start(wk_sb, wk.ap())
                wq_sb = w_pool.tile([P, KO, DG], bf16)
                nc.sync.dma_start(wq_sb, wq.ap())
                # cos/sin live in the rope pool (NOT w_pool): their last
                # readers are the final rope muls, and the attention-phase
                # SBUF pools fit inside w_pool's zone alone, so keeping
                # cos/sin out of w_pool lets those pools allocate as soon
                # as the last Q matmul (w_pool's true last reader) retires.
                cos_sb = rope.tile([P, S], bf16, tag="cos", bufs=1)
                nc.sync.dma_start(cos_sb, cosT.ap())
                sin_sb = rope.tile([P, S], bf16, tag="sin", bufs=1)
                nc.sync.dma_start(sin_sb, sinT.ap())
                nc.sync.dma_start(mask_sb, maskD.ap())

                def p1tile(last):
                    if last:
                        return ps1b.tile([P, QT], fp32, tag="psvb",
                                         name="psvb", bufs=2)
                    return ps1a.tile([P, QT], fp32, tag="psva",
                                     name="psva", bufs=6)

                # V: [keys, dg] natural layout. k-outer over two passes of
                # 8 m-groups (8 PSUM banks) so each arriving (wv, x) chunk
                # pair is consumed by 8 matmuls immediately.
                for half in range(2):
                    ms = list(range(8 * half, 8 * half + 8))
                    pss = {}
                    for m in ms:
                        pss[m] = p1tile(m % 8 >= 6)
                    for k in range(KO):
                        for m in ms:
                            nc.tensor.matmul(
                                pss[m], xT_sb[:, k, m * P:(m + 1) * P],
                                wv_sb[:, k, :],
                                start=(k == 0), stop=(k == KO - 1))
                    for m in ms:
                        nc.scalar.copy(v_sb[:, m, :], pss[m])

                # K then Q: [HD, S] transposed layout + RoPE.
                # Heads processed in pairs; psum groups rotate the 6-deep
                # ps1a ring so RoPE of one pair overlaps the next matmuls.
                for which, w_sb, dstT in (("k", wk_sb, kT_sb), ("q", wq_sb, qT_sb)):
                    for nt2 in range(2 * NQT):
                        nt, hp = divmod(nt2, 2)
                        sl = slice(nt * QT, (nt + 1) * QT)
                        heads = (2 * hp, 2 * hp + 1)
                        last_pair = which == "q" and nt2 == 2 * NQT - 1
                        pss = {}
                        for h in heads:
                            pss[h] = p1tile(last_pair)
                        for k in range(KO):
                            for h in heads:
                                nc.tensor.matmul(
                                    pss[h], w_sb[:, k, h * HD:(h + 1) * HD],
                                    xT_sb[:, k, sl],
                                    start=(k == 0), stop=(k == KO - 1))
                        for h in heads:
                            ps = pss[h]
                            dst = dstT[:, h, sl]
                            # rope: dst = ps * cos + swap(ps) * sin_signed.
                            # The swapped reads must come from PSUM (the SB-SB
                            # same-base-partition rule forbids them on SBUF);
                            # the straight read goes via a parallel ACT copy so
                            # the psum bank drains fast.
                            tmp = rope.tile([P, QT], bf16, tag="tmp")
                            nc.vector.tensor_mul(tmp[0:64], ps[64:128],
                                                 sin_sb[0:64, sl])
                            nc.vector.tensor_mul(tmp[64:128], ps[0:64],
                                                 sin_sb[64:128, sl])
                            qb = rope.tile([P, QT], bf16, tag="qb")
                            nc.scalar.copy(qb, ps)
                            nc.vector.tensor_mul(dst, qb, cos_sb[:, sl])
                            nc.vector.tensor_add(dst, dst, tmp)

            # ---------------- phases 2+3 ----------------
            with tc.tile_pool(name="big2", bufs=1) as big2:
                wo_sb = big2.tile([P, NH, D], bf16)
                nc.sync.dma_start(wo_sb, wo.ap())

                with tc.tile_pool(name="ax_pool", bufs=10) as ax_pool, \
                     tc.tile_pool(name="accp", bufs=3) as accp, \
                     tc.tile_pool(name="ep", bufs=3) as ep, \
                     tc.tile_pool(name="stage", bufs=6) as stage, \
                     tc.tile_pool(name="ps2s", bufs=3, space="PSUM") as ps2s, \
                     tc.tile_pool(name="ps2o", bufs=2, space="PSUM") as ps2o, \
                     tc.tile_pool(name="ps2", bufs=1, space="PSUM") as ps2, \
                     tc.tile_pool(name="ps3", bufs=2, space="PSUM") as ps3:

                    drain_ct = 0

                    def outproj_qo(qo):
                        nonlocal drain_ct
                        for n in range(D // QT):
                            nsl = slice(n * QT, (n + 1) * QT)
                            ps = ps3.tile([P, QT], fp32, tag="ps_out")
                            for h in range(NH):
                                nc.tensor.matmul(
                                    ps, aoT_sb[:, h, qo * P:(qo + 1) * P],
                                    wo_sb[:, h, nsl],
                                    start=(h == 0), stop=(h == NH - 1))
                            ob = stage.tile([P, QT], fp16, tag="ob")
                            # alternate the psum drains across ACT/DVE
                            if drain_ct % 2 == 0:
                                nc.scalar.copy(ob, ps)
                            else:
                                nc.vector.tensor_copy(ob, ps)
                            drain_ct += 1
                            nc.sync.dma_start(out.ap()[:, qo, nsl], ob)

                    def emit_tail(t):
                        # denominator: one ones-matmul over the
                        # chain-accumulated exp sums, then normalize
                        qstart, w, h, ps_o, acc = t
                        ps_sum = ps2.tile([P, QT], fp32, tag="ps_sum")
                        nc.tensor.matmul(ps_sum[:, 0:w], ones, acc[:, 0:w],
                                         start=True, stop=True)
                        rec = ep.tile([P, QT], fp32, tag="rec")
                        nc.vector.reciprocal_approx_fast(rec[:, 0:w],
                                                         ps_sum[:, 0:w])
                        nc.vector.tensor_mul(
                            aoT_sb[:, h, qstart:qstart + w],
                            ps_o[:, 0:w], rec[:, 0:w])

                    # Each head's softmax tail (ones/reciprocal/normalize)
                    # and one deferred out-projection row block are emitted
                    # behind the NEXT head's first chunk, so the PE never
                    # waits on the DVE accumulate chain and the psum drains
                    # spread thinly over the ACT/DVE queues.
                    tail = None
                    pending = []  # deferred out-projection row blocks (qo)
                    for qstart, w in BLOCKS:
                        n_kt = (qstart + w) // P
                        for h in range(NH):
                            ps_o = ps2o.tile([P, QT], fp32, tag="ps_o")
                            acc = accp.tile([P, QT], bf16, tag="acc")
                            m = 0
                            while m < n_kt:
                                # exact-causal trim: chunk m only feeds
                                # q columns >= m*P
                                lo = max(0, m * P - qstart)
                                ww = w - lo
                                ps_s = ps2s.tile([P, QT], fp32, tag="ps_s")
                                if (w == 256 and 0 < m and m + 2 < n_kt
                                        and (m + 2) * P <= qstart):
                                    # two full-width non-diagonal chunks
                                    # share one psum bank and ONE exp
                                    # instruction (halves the ACT issue +
                                    # psum-access overhead that paces the
                                    # PE in the narrow blocks)
                                    for i in range(2):
                                        nc.tensor.matmul(
                                            ps_s[:, i * w:(i + 1) * w],
                                            kT_sb[:, h,
                                                  (m + i) * P:(m + i + 1) * P],
                                            qT_sb[:, h, qstart:qstart + w],
                                            start=True, stop=True,
                                            skip_group_check=True)
                                    tgt = ax_pool.tile([P, QT], bf16,
                                                       tag="ax")
                                    nc.scalar.activation(tgt, ps_s,
                                                         Exp, scale=SCALE)
                                    for i in range(2):
                                        nc.tensor.matmul(
                                            ps_o[:, 0:w],
                                            v_sb[:, m + i,
                                                 h * HD:(h + 1) * HD],
                                            tgt[:, i * w:(i + 1) * w],
                                            start=False, stop=False,
                                            skip_group_check=True)
                                        nc.vector.tensor_add(
                                            acc[:, 0:w], acc[:, 0:w],
                                            tgt[:, i * w:(i + 1) * w])
                                    m += 2
                                    continue
                                nc.tensor.matmul(
                                    ps_s[:, lo:lo + ww],
                                    kT_sb[:, h, m * P:(m + 1) * P],
                                    qT_sb[:, h, qstart + lo:qstart + w],
                                    start=True, stop=True)
                                tgt = acc if m == 0 else ax_pool.tile(
                                    [P, QT], bf16, tag="ax")
                                nc.scalar.activation(tgt[:, lo:lo + ww],
                                                     ps_s[:, lo:lo + ww],
                                                     Exp, scale=SCALE)
                                if m * P >= qstart:
                                    # diagonal chunk: triangle-mask the
                                    # first 128 columns it covers
                                    nc.vector.tensor_mul(
                                        tgt[:, lo:lo + P], tgt[:, lo:lo + P],
                                        mask_sb)
                                nc.tensor.matmul(
                                    ps_o[:, lo:lo + ww],
                                    v_sb[:, m, h * HD:(h + 1) * HD],
                                    tgt[:, lo:lo + ww],
                                    start=(m == 0), stop=(m == n_kt - 1),
                                    skip_group_check=True)
                                if m > 0:
                                    nc.vector.tensor_add(acc[:, lo:lo + ww],
                                                         acc[:, lo:lo + ww],
                                                         tgt[:, lo:lo + ww])
                                if m == 0:
                                    if tail is not None:
                                        emit_tail(tail)
                                        tail = None
                                    if pending:
                                        outproj_qo(pending.pop(0))
                                m += 1
                            tail = (qstart, w, h, ps_o, acc)
                        pending.extend(range(qstart // P, (qstart + w) // P))
                    emit_tail(tail)
                    for qo in pending:
                        outproj_qo(qo)

    nc.compile()
    return nc


def _rope_tables():
    inv_freq = 1.0 / (ROPE_THETA ** (np.arange(0, HD, 2, dtype=np.float64) / HD))
    pos = np.arange(S, dtype=np.float64)
    freqs = np.outer(pos, inv_freq)                    # [S, HD/2]
    emb = np.concatenate([freqs, freqs], axis=-1)      # [S, HD]
    cos = np.cos(emb).T.astype(BF16)                   # [HD, S]
    sin = np.sin(emb).T.astype(np.float32)
    sin[: HD // 2] *= -1.0                             # fold rotate_half sign
    return cos, sin.astype(BF16)


def _pack_kd(a):
    """[D, N] -> [P, D//P, N] with d = ko*P + p."""
    d, n = a.shape
    return np.ascontiguousarray(
        a.reshape(d // P, P, n).transpose(1, 0, 2)).astype(BF16)


def make_in_maps(x, wq, wk, wv, wo):
    cosT, sinT = _rope_tables()
    i = np.arange(P)[:, None]
    j = np.arange(P)[None, :]
    mask = (i <= j).astype(BF16)                       # [128, 128] triangle

    in_maps = []
    for c in range(N_CORES):
        b, g = divmod(c, G)
        gsl = slice(g * DG, (g + 1) * DG)
        in_maps.append({
            "xT": _pack_kd(np.ascontiguousarray(x[b].T)),
            "wq": _pack_kd(wq[:, gsl]),
            "wk": _pack_kd(wk[:, gsl]),
            "wv": _pack_kd(wv[:, gsl]),
            "wo": _pack_kd(np.ascontiguousarray(wo[gsl, :])),
            "cosT": cosT,
            "sinT": sinT,
            "maskD": mask,
        })
    return in_maps


def assemble_output(results):
    """results: list of 8 dicts with 'out' [P, NKT, D] fp16."""
    full = np.empty((B, S, D), dtype=np.float32)
    for b in range(B):
        acc = None
        for g in range(G):
            r = results[b * G + g]["out"].astype(np.float32)
            part = r.transpose(1, 0, 2).reshape(S, D)
            acc = part if acc is None else acc + part
        full[b] = acc
    return full


def _get_module():
    global _BUILT
    if _BUILT is None:
        _BUILT = build_module()
    return _BUILT


def _install_trace_shim():
    """This image's antenv lacks axon_hooks; provide the NTFF profile hook
    via ctypes so trace=True (or BASS_TRACE=1) works instead of crashing,
    and skip the artifact bucket upload."""
    try:
        import antenv.axon_hooks  # noqa: F401
        return
    except ImportError:
        pass
    import types
    import ctypes
    import contextlib

    so_path = "/opt/axon/libaxon_pjrt.so"
    mod = types.ModuleType("antenv.axon_hooks")
    try:
        lib = ctypes.CDLL(so_path)
        lib.axon_start_nrt_profile.argtypes = [
            ctypes.POINTER(ctypes.c_int64), ctypes.c_size_t]
        lib.axon_start_nrt_profile.restype = ctypes.c_int64
        lib.axon_stop_nrt_profile.argtypes = [ctypes.c_char_p]
        lib.axon_stop_nrt_profile.restype = ctypes.c_int64

        @contextlib.contextmanager
        def _hook(output_dir, device_ids):
            import jax
            jax.devices()
            if device_ids:
                ids = (ctypes.c_int64 * len(device_ids))(*device_ids)
                rc = lib.axon_start_nrt_profile(ids, len(device_ids))
            else:
                rc = lib.axon_start_nrt_profile(None, 0)
            if rc != 0:
                raise RuntimeError(f"axon_start_nrt_profile rc={rc}")
            try:
                yield
            finally:
                lib.axon_stop_nrt_profile(str(output_dir).encode())

        mod.get_axon_ntff_profile_hook = lambda: _hook
    except OSError:
        mod.get_axon_ntff_profile_hook = lambda: None
    mod.set_axon_ntff_profile_hook = lambda h: None
    sys.modules["antenv.axon_hooks"] = mod

    from concourse import bass_utils
    bass_utils.upload_artifacts = lambda tmpdir: tmpdir


def run_on_hw(in_maps, trace=False, trace_cores=None):
    _install_trace_shim()
    from concourse import bass_utils
    nc = _get_module()
    return bass_utils.run_bass_kernel_spmd(
        nc, in_maps, core_ids=list(range(N_CORES)),
        trace=trace, trace_cores=trace_cores)


def kernel(x, wq, wk, wv, wo):
    x = np.asarray(x, dtype=np.float32)
    wq = np.asarray(wq, dtype=np.float32)
    wk = np.asarray(wk, dtype=np.float32)
    wv = np.asarray(wv, dtype=np.float32)
    wo = np.asarray(wo, dtype=np.float32)
    in_maps = make_in_maps(x, wq, wk, wv, wo)
    res = run_on_hw(in_maps, trace=False)
    return assemble_output(res.results)


# revision 21
# speedup vs baseline: 1.0212x; 1.0066x over previous
"""Llama attention layer (B=2, S=2048, D=2048, H=16, HD=128, RoPE, causal)
on 8 Trainium2 NeuronCores.

Sharding: core c -> (batch b = c//4, head group g = c%4 of 4 heads).
Each core computes q/k/v projections for its 512 columns of wq/wk/wv,
RoPE, causal attention for its 4 heads, and the out-projection against
its 512 rows of wo (a partial sum over head groups). The host sums the
4 partials per batch and stacks the 2 batches.

All device matmuls run in bf16 with fp32 PSUM accumulation (fp8 was
measured on-device: DoubleRow fp8 is exactly 2x bf16 MACs/cycle, but
plain fp8 quantization costs 3-5e-2 relative error — over the 2e-2
budget — and residual-corrected fp8 needs 3 half-rate GEMM terms =
1.5x bf16 time, so bf16 is optimal here). Softmax is computed without
max-subtraction (scores here are bounded ~|9|).

v2 changes over the first working version (352.8us -> ~329us):
- V-projection runs k-outer / m-inner in two passes of 8 PSUM groups so
  the first matmuls start as soon as x chunk 0 lands (DMA cascade of
  interleaved wv/x chunk transfers) instead of ~16us in.
- Exact-causal trimming: score/AV matmuls and the exp only cover
  q >= key-chunk-start column ranges (53% of the S x S square instead
  of the 62.5% block-causal coverage), with a single shared [128,128]
  triangle mask.
- Softmax denominator: exp chunks are chain-accumulated on DVE in bf16
  and reduced by ONE ones-matmul per (block, head) instead of one per
  chunk pair (saves ~33k PE cycles/core).
- Phase-1 PSUM is split 6+2 banks (ps1a/ps1b) with the last Q head-pair
  on ps1b, so the attention pools' allocation (pool-release WAR) waits
  only on ps1a — released when the last Q matmul retires — instead of
  on the final rope drains (+3.3us).
- Each head's softmax tail (ones/reciprocal/normalize) and one
  out-projection row block are deferred past the NEXT head's first
  chunk, so the PE never stalls on the DVE accumulate chain and psum
  drains spread thinly over the ACT/DVE queues instead of bursting.
- The final 512 q columns are processed as two 256-col blocks so the
  last out-projections have less attention serialized ahead of them;
  within those blocks, pairs of full-width non-diagonal chunks share
  one psum bank and ONE exp instruction (halving the ACT issue +
  psum-access overhead that paces the PE there).
- Output is written as fp16 partials (halves output DMA bytes; host
  sums partials in fp32).
"""

import os
import sys

import numpy as np
import ml_dtypes

if "/opt/trn_rl_repo" not in sys.path:
    sys.path.insert(0, "/opt/trn_rl_repo")

import concourse.bass as bass  # noqa: E402
import concourse.mybir as mybir  # noqa: E402
import concourse.bacc as bacc  # noqa: E402
import concourse.tile as tile  # noqa: E402

BF16 = ml_dtypes.bfloat16

B, S, D, H = 2, 2048, 2048, 16
HD = D // H            # 128, head dim
G = 4                  # head groups (cores per batch)
NH = H // G            # 4 heads per core
DG = NH * HD           # 512, per-core head width
P = 128
KO = D // P            # 16 k-subtiles over D
NKT = S // P           # 16 key chunks of 128
NQT = S // 512         # 4 q tiles of 512
QT = 512
ROPE_THETA = 10000.0
SCALE = 1.0 / float(np.sqrt(HD))

# q-column blocks for the attention phase: (start, width).
# The last 512 columns are split so the final out-projections have less
# attention work serialized in front of them (shorter kernel tail).
BLOCKS = [(0, 512), (512, 512), (1024, 512), (1536, 256), (1792, 256)]

N_CORES = 8

_BUILT = None  # (nc,) cache


def build_module():
    fp32 = mybir.dt.float32
    fp16 = mybir.dt.float16
    bf16 = mybir.dt.bfloat16

    nc = bacc.Bacc("TRN2", target_bir_lowering=False, debug=False,
                   num_devices=N_CORES, num_swdge_queues=4)

    xT = nc.dram_tensor("xT", [P, KO, S], bf16, kind="ExternalInput")
    wq = nc.dram_tensor("wq", [P, KO, DG], bf16, kind="ExternalInput")
    wk = nc.dram_tensor("wk", [P, KO, DG], bf16, kind="ExternalInput")
    wv = nc.dram_tensor("wv", [P, KO, DG], bf16, kind="ExternalInput")
    wo = nc.dram_tensor("wo", [P, NH, D], bf16, kind="ExternalInput")
    cosT = nc.dram_tensor("cosT", [P, S], bf16, kind="ExternalInput")
    sinT = nc.dram_tensor("sinT", [P, S], bf16, kind="ExternalInput")
    maskD = nc.dram_tensor("maskD", [P, P], bf16, kind="ExternalInput")
    out = nc.dram_tensor("out", [P, NKT, D], fp16, kind="ExternalOutput")

    Exp = mybir.ActivationFunctionType.Exp

    with tile.TileContext(nc) as tc:
        with tc.tile_pool(name="const", bufs=1) as const, \
             tc.tile_pool(name="big", bufs=1) as big:
            ones = const.tile([P, P], bf16)
            nc.vector.memset(ones, 1.0)
            mask_sb = const.tile([P, P], bf16)
            # dummy exp so the ACT Exp table loads during the DMA prefix,
            # not at the first real exp in the attention phase
            warm = const.tile([1, 1], fp32)
            nc.scalar.activation(warm, ones[0:1, 0:1],
                                 mybir.ActivationFunctionType.Exp)

            qT_sb = big.tile([P, NH, S], bf16)   # per head: [HD, S]
            kT_sb = big.tile([P, NH, S], bf16)
            v_sb = big.tile([P, NKT, DG], bf16)  # [key%128, keychunk, dg]
            aoT_sb = big.tile([P, NH, S], bf16)  # attention out^T

            # ---------------- phase 1: projections + RoPE ----------------
            # PSUM is split into a 6-bank pool (ps1a) and a 2-bank pool
            # (ps1b). Everything phase 1 does lands on ps1a except the V
            # groups m6/7/14/15 and the LAST Q head-pair, which go to ps1b.
            # The attention-phase pools for scores/ps_o/sum reuse only
            # ps1a's banks, so their allocation waits on ps1a's release
            # (done as the last Q matmuls retire) and NOT on the final
            # rope drains that hold ps1b — those only gate the out-proj
            # pool, which is first needed several microseconds later.
            with tc.tile_pool(name="w_pool", bufs=1) as w_pool, \
                 tc.tile_pool(name="rope", bufs=4) as rope, \
                 tc.tile_pool(name="ps1a", bufs=1, space="PSUM") as ps1a, \
                 tc.tile_pool(name="ps1b", bufs=1, space="PSUM") as ps1b:
                # DMA cascade: per-k interleaved wv/x chunks so the V loop
                # (k-outer) can start as soon as chunk 0 lands.
                wv_sb = w_pool.tile([P, KO, DG], bf16)
                xT_sb = w_pool.tile([P, KO, S], bf16)
                for k in range(KO):
                    nc.sync.dma_start(wv_sb[:, k, :], wv.ap()[:, k, :])
                    nc.sync.dma_start(xT_sb[:, k, :], xT.ap()[:, k, :])
                wk_sb = w_pool.tile([P, KO, DG], bf16)
                nc.sync.dma_start(wk_sb, wk.ap())
                wq_sb = w_pool.tile([P, KO, DG], bf16)
                nc.sync.dma_start(wq_sb, wq.ap())
                # cos/sin live in the rope pool (NOT w_pool): their last
                # readers are the final rope muls, and the attention-phase
                # SBUF pools fit inside w_pool's zone alone, so keeping
                # cos/sin out of w_pool lets those pools allocate as soon
                # as the last Q matmul (w_pool's true last reader) retires.
                cos_sb = rope.tile([P, S], bf16, tag="cos", bufs=1)
                nc.sync.dma_start(cos_sb, cosT.ap())
                sin_sb = rope.tile([P, S], bf16, tag="sin", bufs=1)
                nc.sync.dma_start(sin_sb, sinT.ap())
                nc.sync.dma_start(mask_sb, maskD.ap())

                def p1tile(last):
                    if last:
                        return ps1b.tile([P, QT], fp32, tag="psvb",
                                         name="psvb", bufs=2)
                    return ps1a.tile([P, QT], fp32, tag="psva",
                                     name="psva", bufs=6)

                # V: [keys, dg] natural layout. k-outer over two passes of
                # 8 m-groups (8 PSUM banks) so each arriving (wv, x) chunk
                # pair is consumed by 8 matmuls immediately.
                for half in range(2):
                    ms = list(range(8 * half, 8 * half + 8))
                    pss = {}
                    for m in ms:
                        pss[m] = p1tile(m % 8 >= 6)
                    for k in range(KO):
                        for m in ms:
                            nc.tensor.matmul(
                                pss[m], xT_sb[:, k, m * P:(m + 1) * P],
                                wv_sb[:, k, :],
                                start=(k == 0), stop=(k == KO - 1))
                    for m in ms:
                        nc.scalar.copy(v_sb[:, m, :], pss[m])

                # K then Q: [HD, S] transposed layout + RoPE.
                # Heads processed in pairs; psum groups rotate the 6-deep
                # ps1a ring so RoPE of one pair overlaps the next matmuls.
                for which, w_sb, dstT in (("k", wk_sb, kT_sb), ("q", wq_sb, qT_sb)):
                    for nt2 in range(2 * NQT):
                        nt, hp = divmod(nt2, 2)
                        sl = slice(nt * QT, (nt + 1) * QT)
                        heads = (2 * hp, 2 * hp + 1)
                        last_pair = which == "q" and nt2 == 2 * NQT - 1
                        pss = {}
                        for h in heads:
                            pss[h] = p1tile(last_pair)
                        for k in range(KO):
                            for h in heads:
                                nc.tensor.matmul(
                                    pss[h], w_sb[:, k, h * HD:(h + 1) * HD],
                                    xT_sb[:, k, sl],
                                    start=(k == 0), stop=(k == KO - 1))
                        for h in heads:
                            ps = pss[h]
                            dst = dstT[:, h, sl]
                            # rope: dst = ps * cos + swap(ps) * sin_signed.
                            # The swapped reads must come from PSUM (the SB-SB
                            # same-base-partition rule forbids them on SBUF);
                            # the straight read goes via a parallel ACT copy so
                            # the psum bank drains fast.
                            tmp = rope.tile([P, QT], bf16, tag="tmp")
                            nc.vector.tensor_mul(tmp[0:64], ps[64:128],
                                                 sin_sb[0:64, sl])
                            nc.vector.tensor_mul(tmp[64:128], ps[0:64],
                                                 sin_sb[64:128, sl])
                            qb = rope.tile([P, QT], bf16, tag="qb")
                            nc.scalar.copy(qb, ps)
                            nc.vector.tensor_mul(dst, qb, cos_sb[:, sl])
                            nc.vector.tensor_add(dst, dst, tmp)

            # ---------------- phases 2+3 ----------------
            with tc.tile_pool(name="big2", bufs=1) as big2:
                wo_sb = big2.tile([P, NH, D], bf16)
                nc.sync.dma_start(wo_sb, wo.ap())

                with tc.tile_pool(name="ax_pool", bufs=10) as ax_pool, \
                     tc.tile_pool(name="accp", bufs=3) as accp, \
                     tc.tile_pool(name="ep", bufs=3) as ep, \
                     tc.tile_pool(name="stage", bufs=6) as stage, \
                     tc.tile_pool(name="ps2s", bufs=3, space="PSUM") as ps2s, \
                     tc.tile_pool(name="ps2o", bufs=2, space="PSUM") as ps2o, \
                     tc.tile_pool(name="ps2", bufs=1, space="PSUM") as ps2, \
                     tc.tile_pool(name="ps3", bufs=2, space="PSUM") as ps3:

                    drain_ct = 0

                    def outproj_qo(qo):
                        nonlocal drain_ct
                        for n in range(D // QT):
                            nsl = slice(n * QT, (n + 1) * QT)
                            ps = ps3.tile([P, QT], fp32, tag="ps_out")
                            for h in range(NH):
                                nc.tensor.matmul(
                                    ps, aoT_sb[:, h, qo * P:(qo + 1) * P],
                                    wo_sb[:, h, nsl],
                                    start=(h == 0), stop=(h == NH - 1))
                            ob = stage.tile([P, QT], fp16, tag="ob")
                            # alternate the psum drains across ACT/DVE
                            if drain_ct % 2 == 0:
                                nc.scalar.copy(ob, ps)
                            else:
                                nc.vector.tensor_copy(ob, ps)
                            drain_ct += 1
                            nc.sync.dma_start(out.ap()[:, qo, nsl], ob)

                    def emit_tail(t):
                        # denominator: one ones-matmul over the
                        # chain-accumulated exp sums, then normalize
                        qstart, w, h, ps_o, acc = t
                        ps_sum = ps2.tile([P, QT], fp32, tag="ps_sum")
                        nc.tensor.matmul(ps_sum[:, 0:w], ones, acc[:, 0:w],
                                         start=True, stop=True)
                        rec = ep.tile([P, QT], fp32, tag="rec")
                        nc.vector.reciprocal_approx_fast(rec[:, 0:w],
                                                         ps_sum[:, 0:w])
                        nc.vector.tensor_mul(
                            aoT_sb[:, h, qstart:qstart + w],
                            ps_o[:, 0:w], rec[:, 0:w])

                    # Each head's softmax tail (ones/reciprocal/normalize)
                    # and one deferred out-projection row block are emitted
                    # behind the NEXT head's first chunk, so the PE never
                    # waits on the DVE accumulate chain and the psum drains
                    # spread thinly over the ACT/DVE queues.
                    tail = None
                    pending = []  # deferred out-projection row blocks (qo)
                    for qstart, w in BLOCKS:
                        n_kt = (qstart + w) // P
                        for h in range(NH):
                            ps_o = ps2o.tile([P, QT], fp32, tag="ps_o")
                            acc = accp.tile([P, QT], bf16, tag="acc")
                            m = 0
                            while m < n_kt:
                                # exact-causal trim: chunk m only feeds
                                # q columns >= m*P
                                lo = max(0, m * P - qstart)
                                ww = w - lo
                                ps_s = ps2s.tile([P, QT], fp32, tag="ps_s")
                                ww_nxt = w - max(0, (m + 1) * P - qstart)
                                if 0 < m and m + 1 < n_kt and ww + ww_nxt <= QT:
                                    # two adjacent chunks (trimmed widths
                                    # fit one bank) share one psum tile,
                                    # packed side by side, and ONE exp
                                    # instruction — halves the ACT issue +
                                    # psum-access overhead that paces the
                                    # PE. Chunk data is relocated within
                                    # the tile; AV/mask/add use the right
                                    # block-column slices regardless.
                                    mems = []
                                    off = 0
                                    for mm in (m, m + 1):
                                        lo2 = max(0, mm * P - qstart)
                                        ww2 = w - lo2
                                        mems.append(
                                            (mm, lo2, ww2, off,
                                             mm * P >= qstart))
                                        off += ww2
                                    for mm, lo2, ww2, o, dg in mems:
                                        nc.tensor.matmul(
                                            ps_s[:, o:o + ww2],
                                            kT_sb[:, h, mm * P:(mm + 1) * P],
                                            qT_sb[:, h,
                                                  qstart + lo2:qstart + w],
                                            start=True, stop=True,
                                            skip_group_check=True)
                                    tgt = ax_pool.tile([P, QT], bf16,
                                                       tag="ax")
                                    nc.scalar.activation(tgt[:, 0:off],
                                                         ps_s[:, 0:off],
                                                         Exp, scale=SCALE)
                                    for mm, lo2, ww2, o, dg in mems:
                                        if dg:
                                            nc.vector.tensor_mul(
                                                tgt[:, o:o + P],
                                                tgt[:, o:o + P], mask_sb)
                                        nc.tensor.matmul(
                                            ps_o[:, lo2:lo2 + ww2],
                                            v_sb[:, mm, h * HD:(h + 1) * HD],
                                            tgt[:, o:o + ww2],
                                            start=False,
                                            stop=(mm == n_kt - 1),
                                            skip_group_check=True)
                                        nc.vector.tensor_add(
                                            acc[:, lo2:lo2 + ww2],
                                            acc[:, lo2:lo2 + ww2],
                                            tgt[:, o:o + ww2])
                                    m += 2
                                    continue
                                nc.tensor.matmul(
                                    ps_s[:, lo:lo + ww],
                                    kT_sb[:, h, m * P:(m + 1) * P],
                                    qT_sb[:, h, qstart + lo:qstart + w],
                                    start=True, stop=True)
                                tgt = acc if m == 0 else ax_pool.tile(
                                    [P, QT], bf16, tag="ax")
                                nc.scalar.activation(tgt[:, lo:lo + ww],
                                                     ps_s[:, lo:lo + ww],
                                                     Exp, scale=SCALE)
                                if m * P >= qstart:
                                    # diagonal chunk: triangle-mask the
                                    # first 128 columns it covers
                                    nc.vector.tensor_mul(
                                        tgt[:, lo:lo + P], tgt[:, lo:lo + P],
                                        mask_sb)
                                nc.tensor.matmul(
                                    ps_o[:, lo:lo + ww],
                                    v_sb[:, m, h * HD:(h + 1) * HD],
                                    tgt[:, lo:lo + ww],
                                    start=(m == 0), stop=(m == n_kt - 1),
                                    skip_group_check=True)
                                if m > 0:
                                    nc.vector.tensor_add(acc[:, lo:lo + ww],
                                                         acc[:, lo:lo + ww],
                                                         tgt[:, lo:lo + ww])
                                if m == 0:
                                    if tail is not None:
                                        emit_tail(tail)
                                        tail = None
                                    if pending:
                                        outproj_qo(pending.pop(0))
                                m += 1
                            tail = (qstart, w, h, ps_o, acc)
                        pending.extend(range(qstart // P, (qstart + w) // P))
                    emit_tail(tail)
                    for qo in pending:
                        outproj_qo(qo)

    nc.compile()
    return nc


def _rope_tables():
    inv_freq = 1.0 / (ROPE_THETA ** (np.arange(0, HD, 2, dtype=np.float64) / HD))
    pos = np.arange(S, dtype=np.float64)
    freqs = np.outer(pos, inv_freq)                    # [S, HD/2]
    emb = np.concatenate([freqs, freqs], axis=-1)      # [S, HD]
    cos = np.cos(emb).T.astype(BF16)                   # [HD, S]
    sin = np.sin(emb).T.astype(np.float32)
    sin[: HD // 2] *= -1.0                             # fold rotate_half sign
    return cos, sin.astype(BF16)


def _pack_kd(a):
    """[D, N] -> [P, D//P, N] with d = ko*P + p."""
    d, n = a.shape
    return np.ascontiguousarray(
        a.reshape(d // P, P, n).transpose(1, 0, 2)).astype(BF16)


def make_in_maps(x, wq, wk, wv, wo):
    cosT, sinT = _rope_tables()
    i = np.arange(P)[:, None]
    j = np.arange(P)[None, :]
    mask = (i <= j).astype(BF16)                       # [128, 128] triangle

    in_maps = []
    for c in range(N_CORES):
        b, g = divmod(c, G)
        gsl = slice(g * DG, (g + 1) * DG)
        in_maps.append({
            "xT": _pack_kd(np.ascontiguousarray(x[b].T)),
            "wq": _pack_kd(wq[:, gsl]),
            "wk": _pack_kd(wk[:, gsl]),
            "wv": _pack_kd(wv[:, gsl]),
            "wo": _pack_kd(np.ascontiguousarray(wo[gsl, :])),
            "cosT": cosT,
            "sinT": sinT,
            "maskD": mask,
        })
    return in_maps


def assemble_output(results):
    """results: list of 8 dicts with 'out' [P, NKT, D] fp16."""
    full = np.empty((B, S, D), dtype=np.float32)
    for b in range(B):
        acc = None
        for g in range(G):
            r = results[b * G + g]["out"].astype(np.float32)
            part = r.transpose(1, 0, 2).reshape(S, D)
            acc = part if acc is None else acc + part
        full[b] = acc
    return full


def _get_module():
    global _BUILT
    if _BUILT is None:
        _BUILT = build_module()
    return _BUILT


def _install_trace_shim():
    """This image's antenv lacks axon_hooks; provide the NTFF profile hook
    via ctypes so trace=True (or BASS_TRACE=1) works instead of crashing,
    and skip the artifact bucket upload."""
    try:
        import antenv.axon_hooks  # noqa: F401
        return
    except ImportError:
        pass
    import types
    import ctypes
    import contextlib

    so_path = "/opt/axon/libaxon_pjrt.so"
    mod = types.ModuleType("antenv.axon_hooks")
    try:
        lib = ctypes.CDLL(so_path)
        lib.axon_start_nrt_profile.argtypes = [
            ctypes.POINTER(ctypes.c_int64), ctypes.c_size_t]
        lib.axon_start_nrt_profile.restype = ctypes.c_int64
        lib.axon_stop_nrt_profile.argtypes = [ctypes.c_char_p]
        lib.axon_stop_nrt_profile.restype = ctypes.c_int64

        @contextlib.contextmanager
        def _hook(output_dir, device_ids):
            import jax
            jax.devices()
            if device_ids:
                ids = (ctypes.c_int64 * len(device_ids))(*device_ids)
                rc = lib.axon_start_nrt_profile(ids, len(device_ids))
            else:
                rc = lib.axon_start_nrt_profile(None, 0)
            if rc != 0:
                raise RuntimeError(f"axon_start_nrt_profile rc={rc}")
            try:
                yield
            finally:
                lib.axon_stop_nrt_profile(str(output_dir).encode())

        mod.get_axon_ntff_profile_hook = lambda: _hook
    except OSError:
        mod.get_axon_ntff_profile_hook = lambda: None
    mod.set_axon_ntff_profile_hook = lambda h: None
    sys.modules["antenv.axon_hooks"] = mod

    from concourse import bass_utils
    bass_utils.upload_artifacts = lambda tmpdir: tmpdir


def run_on_hw(in_maps, trace=False, trace_cores=None):
    _install_trace_shim()
    from concourse import bass_utils
    nc = _get_module()
    return bass_utils.run_bass_kernel_spmd(
        nc, in_maps, core_ids=list(range(N_CORES)),
        trace=trace, trace_cores=trace_cores)


def kernel(x, wq, wk, wv, wo):
    x = np.asarray(x, dtype=np.float32)
    wq = np.asarray(wq, dtype=np.float32)
    wk = np.asarray(wk, dtype=np.float32)
    wv = np.asarray(wv, dtype=np.float32)
    wo = np.asarray(wo, dtype=np.float32)
    in_maps = make_in_maps(x, wq, wk, wv, wo)
    res = run_on_hw(in_maps, trace=False)
    return assemble_output(res.results)


# revision 23
# speedup vs baseline: 1.0355x; 1.0140x over previous
"""Llama attention layer (B=2, S=2048, D=2048, H=16, HD=128, RoPE, causal)
on 8 Trainium2 NeuronCores.

Sharding: core c -> (batch b = c//4, head group g = c%4 of 4 heads).
Each core computes q/k/v projections for its 512 columns of wq/wk/wv,
RoPE, causal attention for its 4 heads, and the out-projection against
its 512 rows of wo (a partial sum over head groups). The host sums the
4 partials per batch and stacks the 2 batches.

All device matmuls run in bf16 with fp32 PSUM accumulation (fp8 was
measured on-device: DoubleRow fp8 is exactly 2x bf16 MACs/cycle, but
plain fp8 quantization costs 3-5e-2 relative error — over the 2e-2
budget — and residual-corrected fp8 needs 3 half-rate GEMM terms =
1.5x bf16 time, so bf16 is optimal here). Softmax is computed without
max-subtraction (scores here are bounded ~|9|).

v2 changes over the first working version (352.8us -> ~329us):
- V-projection runs k-outer / m-inner in two passes of 8 PSUM groups so
  the first matmuls start as soon as x chunk 0 lands (DMA cascade of
  interleaved wv/x chunk transfers) instead of ~16us in.
- Exact-causal trimming: score/AV matmuls and the exp only cover
  q >= key-chunk-start column ranges (53% of the S x S square instead
  of the 62.5% block-causal coverage), with a single shared [128,128]
  triangle mask.
- Softmax denominator: exp chunks are chain-accumulated on DVE in bf16
  and reduced by ONE ones-matmul per (block, head) instead of one per
  chunk pair (saves ~33k PE cycles/core).
- Phase-1 PSUM is split 6+2 banks (ps1a/ps1b) with the last Q head-pair
  on ps1b, so the attention pools' allocation (pool-release WAR) waits
  only on ps1a — released when the last Q matmul retires — instead of
  on the final rope drains (+3.3us).
- Each head's softmax tail (ones/reciprocal/normalize) and one
  out-projection row block are deferred past the NEXT head's first
  chunk, so the PE never stalls on the DVE accumulate chain and psum
  drains spread thinly over the ACT/DVE queues instead of bursting.
- The final 512 q columns are processed as two 256-col blocks so the
  last out-projections have less attention serialized ahead of them.
- Any two adjacent key chunks whose causal-trimmed widths fit one psum
  bank are packed side by side into a single [128,512] tile and share
  ONE exp instruction (cuts ~60 ACT issue + psum-access overheads from
  the exp stream that paces the PE); AV/mask/add address the packed
  sub-slices.
- Output is written as fp16 partials (halves output DMA bytes; host
  sums partials in fp32).
"""

import os
import sys

import numpy as np
import ml_dtypes

if "/opt/trn_rl_repo" not in sys.path:
    sys.path.insert(0, "/opt/trn_rl_repo")

import concourse.bass as bass  # noqa: E402
import concourse.mybir as mybir  # noqa: E402
import concourse.bacc as bacc  # noqa: E402
import concourse.tile as tile  # noqa: E402

BF16 = ml_dtypes.bfloat16

B, S, D, H = 2, 2048, 2048, 16
HD = D // H            # 128, head dim
G = 4                  # head groups (cores per batch)
NH = H // G            # 4 heads per core
DG = NH * HD           # 512, per-core head width
P = 128
KO = D // P            # 16 k-subtiles over D
NKT = S // P           # 16 key chunks of 128
NQT = S // 512         # 4 q tiles of 512
QT = 512
ROPE_THETA = 10000.0
SCALE = 1.0 / float(np.sqrt(HD))

# q-column blocks for the attention phase: (start, width).
# The last 512 columns are split so the final out-projections have less
# attention work serialized in front of them (shorter kernel tail).
BLOCKS = [(0, 512), (512, 512), (1024, 512), (1536, 256), (1792, 256)]

N_CORES = 8

_BUILT = None  # (nc,) cache


def build_module():
    fp32 = mybir.dt.float32
    fp16 = mybir.dt.float16
    bf16 = mybir.dt.bfloat16

    nc = bacc.Bacc("TRN2", target_bir_lowering=False, debug=False,
                   num_devices=N_CORES, num_swdge_queues=4)

    xT = nc.dram_tensor("xT", [P, KO, S], bf16, kind="ExternalInput")
    wq = nc.dram_tensor("wq", [P, KO, DG], bf16, kind="ExternalInput")
    wk = nc.dram_tensor("wk", [P, KO, DG], bf16, kind="ExternalInput")
    wv = nc.dram_tensor("wv", [P, KO, DG], bf16, kind="ExternalInput")
    wo = nc.dram_tensor("wo", [P, NH, D], bf16, kind="ExternalInput")
    cosT = nc.dram_tensor("cosT", [P, S], bf16, kind="ExternalInput")
    sinT = nc.dram_tensor("sinT", [P, S], bf16, kind="ExternalInput")
    maskD = nc.dram_tensor("maskD", [P, P], bf16, kind="ExternalInput")
    out = nc.dram_tensor("out", [P, NKT, D], fp16, kind="ExternalOutput")

    Exp = mybir.ActivationFunctionType.Exp

    with tile.TileContext(nc) as tc:
        with tc.tile_pool(name="const", bufs=1) as const, \
             tc.tile_pool(name="big", bufs=1) as big:
            ones = const.tile([P, P], bf16)
            nc.vector.memset(ones, 1.0)
            mask_sb = const.tile([P, P], bf16)
            # dummy exp so the ACT Exp table loads during the DMA prefix,
            # not at the first real exp in the attention phase
            warm = const.tile([1, 1], fp32)
            nc.scalar.activation(warm, ones[0:1, 0:1],
                                 mybir.ActivationFunctionType.Exp)

            qT_sb = big.tile([P, NH, S], bf16)   # per head: [HD, S]
            kT_sb = big.tile([P, NH, S], bf16)
            v_sb = big.tile([P, NKT, DG], bf16)  # [key%128, keychunk, dg]
            aoT_sb = big.tile([P, NH, S], bf16)  # attention out^T

            # ---------------- phase 1: projections + RoPE ----------------
            # PSUM is split into a 6-bank pool (ps1a) and a 2-bank pool
            # (ps1b). Everything phase 1 does lands on ps1a except the V
            # groups m6/7/14/15 and the LAST Q head-pair, which go to ps1b.
            # The attention-phase pools for scores/ps_o/sum reuse only
            # ps1a's banks, so their allocation waits on ps1a's release
            # (done as the last Q matmuls retire) and NOT on the final
            # rope drains that hold ps1b — those only gate the out-proj
            # pool, which is first needed several microseconds later.
            with tc.tile_pool(name="w_pool", bufs=1) as w_pool, \
                 tc.tile_pool(name="rope", bufs=4) as rope, \
                 tc.tile_pool(name="ps1a", bufs=1, space="PSUM") as ps1a, \
                 tc.tile_pool(name="ps1b", bufs=1, space="PSUM") as ps1b:
                # DMA cascade: per-k interleaved wv/x chunks so the V loop
                # (k-outer) can start as soon as chunk 0 lands.
                wv_sb = w_pool.tile([P, KO, DG], bf16)
                xT_sb = w_pool.tile([P, KO, S], bf16)
                for k in range(KO):
                    nc.sync.dma_start(wv_sb[:, k, :], wv.ap()[:, k, :])
                    nc.sync.dma_start(xT_sb[:, k, :], xT.ap()[:, k, :])
                wk_sb = w_pool.tile([P, KO, DG], bf16)
                nc.sync.dma_start(wk_sb, wk.ap())
                wq_sb = w_pool.tile([P, KO, DG], bf16)
                nc.sync.dma_start(wq_sb, wq.ap())
                # cos/sin live in the rope pool (NOT w_pool): their last
                # readers are the final rope muls, and the attention-phase
                # SBUF pools fit inside w_pool's zone alone, so keeping
                # cos/sin out of w_pool lets those pools allocate as soon
                # as the last Q matmul (w_pool's true last reader) retires.
                cos_sb = rope.tile([P, S], bf16, tag="cos", bufs=1)
                nc.sync.dma_start(cos_sb, cosT.ap())
                sin_sb = rope.tile([P, S], bf16, tag="sin", bufs=1)
                nc.sync.dma_start(sin_sb, sinT.ap())
                nc.sync.dma_start(mask_sb, maskD.ap())

                def p1tile(last):
                    if last:
                        return ps1b.tile([P, QT], fp32, tag="psvb",
                                         name="psvb", bufs=2)
                    return ps1a.tile([P, QT], fp32, tag="psva",
                                     name="psva", bufs=6)

                # V: [keys, dg] natural layout. k-outer over two passes of
                # 8 m-groups (8 PSUM banks) so each arriving (wv, x) chunk
                # pair is consumed by 8 matmuls immediately.
                for half in range(2):
                    ms = list(range(8 * half, 8 * half + 8))
                    pss = {}
                    for m in ms:
                        pss[m] = p1tile(m % 8 >= 6)
                    for k in range(KO):
                        for m in ms:
                            nc.tensor.matmul(
                                pss[m], xT_sb[:, k, m * P:(m + 1) * P],
                                wv_sb[:, k, :],
                                start=(k == 0), stop=(k == KO - 1))
                    for m in ms:
                        nc.scalar.copy(v_sb[:, m, :], pss[m])

                # K then Q: [HD, S] transposed layout + RoPE.
                # Heads processed in pairs; psum groups rotate the 6-deep
                # ps1a ring so RoPE of one pair overlaps the next matmuls.
                for which, w_sb, dstT in (("k", wk_sb, kT_sb), ("q", wq_sb, qT_sb)):
                    for nt2 in range(2 * NQT):
                        nt, hp = divmod(nt2, 2)
                        sl = slice(nt * QT, (nt + 1) * QT)
                        heads = (2 * hp, 2 * hp + 1)
                        last_pair = which == "q" and nt2 == 2 * NQT - 1
                        pss = {}
                        for h in heads:
                            pss[h] = p1tile(last_pair)
                        for k in range(KO):
                            for h in heads:
                                nc.tensor.matmul(
                                    pss[h], w_sb[:, k, h * HD:(h + 1) * HD],
                                    xT_sb[:, k, sl],
                                    start=(k == 0), stop=(k == KO - 1))
                        for h in heads:
                            ps = pss[h]
                            dst = dstT[:, h, sl]
                            # rope: dst = ps * cos + swap(ps) * sin_signed.
                            # The swapped reads must come from PSUM (the SB-SB
                            # same-base-partition rule forbids them on SBUF);
                            # the straight read goes via a parallel ACT copy so
                            # the psum bank drains fast.
                            tmp = rope.tile([P, QT], bf16, tag="tmp")
                            nc.vector.tensor_mul(tmp[0:64], ps[64:128],
                                                 sin_sb[0:64, sl])
                            nc.vector.tensor_mul(tmp[64:128], ps[0:64],
                                                 sin_sb[64:128, sl])
                            qb = rope.tile([P, QT], bf16, tag="qb")
                            nc.scalar.copy(qb, ps)
                            nc.vector.tensor_mul(dst, qb, cos_sb[:, sl])
                            nc.vector.tensor_add(dst, dst, tmp)

            # ---------------- phases 2+3 ----------------
            with tc.tile_pool(name="big2", bufs=1) as big2:
                wo_sb = big2.tile([P, NH, D], bf16)
                nc.sync.dma_start(wo_sb, wo.ap())

                with tc.tile_pool(name="ax_pool", bufs=10) as ax_pool, \
                     tc.tile_pool(name="accp", bufs=3) as accp, \
                     tc.tile_pool(name="ep", bufs=3) as ep, \
                     tc.tile_pool(name="stage", bufs=6) as stage, \
                     tc.tile_pool(name="ps2s", bufs=3, space="PSUM") as ps2s, \
                     tc.tile_pool(name="ps2o", bufs=2, space="PSUM") as ps2o, \
                     tc.tile_pool(name="ps2", bufs=1, space="PSUM") as ps2, \
                     tc.tile_pool(name="ps3", bufs=2, space="PSUM") as ps3:

                    drain_ct = 0

                    def outproj_qo(qo, pools=None):
                        nonlocal drain_ct
                        for n in range(D // QT):
                            nsl = slice(n * QT, (n + 1) * QT)
                            if pools is None:
                                ps = ps3.tile([P, QT], fp32, tag="ps_out")
                            else:
                                # kernel tail: the attention pools are done,
                                # so rotate the final groups over ALL psum
                                # rings — drains never block the matmuls
                                pool, tg = pools[n % len(pools)]
                                ps = pool.tile([P, QT], fp32, tag=tg,
                                               name="ps_fin")
                            for h in range(NH):
                                nc.tensor.matmul(
                                    ps, aoT_sb[:, h, qo * P:(qo + 1) * P],
                                    wo_sb[:, h, nsl],
                                    start=(h == 0), stop=(h == NH - 1))
                            ob = stage.tile([P, QT], fp16, tag="ob")
                            # alternate the psum drains across ACT/DVE
                            if drain_ct % 2 == 0:
                                nc.scalar.copy(ob, ps)
                            else:
                                nc.vector.tensor_copy(ob, ps)
                            drain_ct += 1
                            nc.sync.dma_start(out.ap()[:, qo, nsl], ob)

                    def emit_tail(t):
                        # denominator: one ones-matmul over the
                        # chain-accumulated exp sums, then normalize
                        qstart, w, h, ps_o, acc = t
                        ps_sum = ps2.tile([P, QT], fp32, tag="ps_sum")
                        nc.tensor.matmul(ps_sum[:, 0:w], ones, acc[:, 0:w],
                                         start=True, stop=True)
                        rec = ep.tile([P, QT], fp32, tag="rec")
                        nc.vector.reciprocal_approx_fast(rec[:, 0:w],
                                                         ps_sum[:, 0:w])
                        nc.vector.tensor_mul(
                            aoT_sb[:, h, qstart:qstart + w],
                            ps_o[:, 0:w], rec[:, 0:w])

                    # Each head's softmax tail (ones/reciprocal/normalize)
                    # and one deferred out-projection row block are emitted
                    # behind the NEXT head's first chunk, so the PE never
                    # waits on the DVE accumulate chain and the psum drains
                    # spread thinly over the ACT/DVE queues.
                    tail = None
                    pending = []  # deferred out-projection row blocks (qo)
                    for qstart, w in BLOCKS:
                        n_kt = (qstart + w) // P
                        for h in range(NH):
                            ps_o = ps2o.tile([P, QT], fp32, tag="ps_o")
                            acc = accp.tile([P, QT], bf16, tag="acc")
                            m = 0
                            while m < n_kt:
                                # exact-causal trim: chunk m only feeds
                                # q columns >= m*P
                                lo = max(0, m * P - qstart)
                                ww = w - lo
                                ps_s = ps2s.tile([P, QT], fp32, tag="ps_s")
                                ww_nxt = w - max(0, (m + 1) * P - qstart)
                                if 0 < m and m + 1 < n_kt and ww + ww_nxt <= QT:
                                    # two adjacent chunks (trimmed widths
                                    # fit one bank) share one psum tile,
                                    # packed side by side, and ONE exp
                                    # instruction — halves the ACT issue +
                                    # psum-access overhead that paces the
                                    # PE. Chunk data is relocated within
                                    # the tile; AV/mask/add use the right
                                    # block-column slices regardless.
                                    mems = []
                                    off = 0
                                    for mm in (m, m + 1):
                                        lo2 = max(0, mm * P - qstart)
                                        ww2 = w - lo2
                                        mems.append(
                                            (mm, lo2, ww2, off,
                                             mm * P >= qstart))
                                        off += ww2
                                    for mm, lo2, ww2, o, dg in mems:
                                        nc.tensor.matmul(
                                            ps_s[:, o:o + ww2],
                                            kT_sb[:, h, mm * P:(mm + 1) * P],
                                            qT_sb[:, h,
                                                  qstart + lo2:qstart + w],
                                            start=True, stop=True,
                                            skip_group_check=True)
                                    tgt = ax_pool.tile([P, QT], bf16,
                                                       tag="ax")
                                    nc.scalar.activation(tgt[:, 0:off],
                                                         ps_s[:, 0:off],
                                                         Exp, scale=SCALE)
                                    for mm, lo2, ww2, o, dg in mems:
                                        if dg:
                                            nc.vector.tensor_mul(
                                                tgt[:, o:o + P],
                                                tgt[:, o:o + P], mask_sb)
                                        nc.tensor.matmul(
                                            ps_o[:, lo2:lo2 + ww2],
                                            v_sb[:, mm, h * HD:(h + 1) * HD],
                                            tgt[:, o:o + ww2],
                                            start=False,
                                            stop=(mm == n_kt - 1),
                                            skip_group_check=True)
                                        nc.vector.tensor_add(
                                            acc[:, lo2:lo2 + ww2],
                                            acc[:, lo2:lo2 + ww2],
                                            tgt[:, o:o + ww2])
                                    m += 2
                                    continue
                                nc.tensor.matmul(
                                    ps_s[:, lo:lo + ww],
                                    kT_sb[:, h, m * P:(m + 1) * P],
                                    qT_sb[:, h, qstart + lo:qstart + w],
                                    start=True, stop=True)
                                tgt = acc if m == 0 else ax_pool.tile(
                                    [P, QT], bf16, tag="ax")
                                nc.scalar.activation(tgt[:, lo:lo + ww],
                                                     ps_s[:, lo:lo + ww],
                                                     Exp, scale=SCALE)
                                if m * P >= qstart:
                                    # diagonal chunk: triangle-mask the
                                    # first 128 columns it covers
                                    nc.vector.tensor_mul(
                                        tgt[:, lo:lo + P], tgt[:, lo:lo + P],
                                        mask_sb)
                                nc.tensor.matmul(
                                    ps_o[:, lo:lo + ww],
                                    v_sb[:, m, h * HD:(h + 1) * HD],
                                    tgt[:, lo:lo + ww],
                                    start=(m == 0), stop=(m == n_kt - 1),
                                    skip_group_check=True)
                                if m > 0:
                                    nc.vector.tensor_add(acc[:, lo:lo + ww],
                                                         acc[:, lo:lo + ww],
                                                         tgt[:, lo:lo + ww])
                                if m == 0:
                                    if tail is not None:
                                        emit_tail(tail)
                                        tail = None
                                    if pending:
                                        outproj_qo(pending.pop(0))
                                m += 1
                            tail = (qstart, w, h, ps_o, acc)
                        pending.extend(range(qstart // P, (qstart + w) // P))
                    emit_tail(tail)
                    fin_pools = [(ps3, "ps_out"), (ps2s, "ps_s"),
                                 (ps2o, "ps_o"), (ps2, "ps_sum")]
                    for qo in pending:
                        outproj_qo(qo, fin_pools)

    nc.compile()
    return nc


def _rope_tables():
    inv_freq = 1.0 / (ROPE_THETA ** (np.arange(0, HD, 2, dtype=np.float64) / HD))
    pos = np.arange(S, dtype=np.float64)
    freqs = np.outer(pos, inv_freq)                    # [S, HD/2]
    emb = np.concatenate([freqs, freqs], axis=-1)      # [S, HD]
    cos = np.cos(emb).T.astype(BF16)                   # [HD, S]
    sin = np.sin(emb).T.astype(np.float32)
    sin[: HD // 2] *= -1.0                             # fold rotate_half sign
    return cos, sin.astype(BF16)


def _pack_kd(a):
    """[D, N] -> [P, D//P, N] with d = ko*P + p."""
    d, n = a.shape
    return np.ascontiguousarray(
        a.reshape(d // P, P, n).transpose(1, 0, 2)).astype(BF16)


def make_in_maps(x, wq, wk, wv, wo):
    cosT, sinT = _rope_tables()
    i = np.arange(P)[:, None]
    j = np.arange(P)[None, :]
    mask = (i <= j).astype(BF16)                       # [128, 128] triangle

    in_maps = []
    for c in range(N_CORES):
        b, g = divmod(c, G)
        gsl = slice(g * DG, (g + 1) * DG)
        in_maps.append({
            "xT": _pack_kd(np.ascontiguousarray(x[b].T)),
            "wq": _pack_kd(wq[:, gsl]),
            "wk": _pack_kd(wk[:, gsl]),
            "wv": _pack_kd(wv[:, gsl]),
            "wo": _pack_kd(np.ascontiguousarray(wo[gsl, :])),
            "cosT": cosT,
            "sinT": sinT,
            "maskD": mask,
        })
    return in_maps


def assemble_output(results):
    """results: list of 8 dicts with 'out' [P, NKT, D] fp16."""
    full = np.empty((B, S, D), dtype=np.float32)
    for b in range(B):
        acc = None
        for g in range(G):
            r = results[b * G + g]["out"].astype(np.float32)
            part = r.transpose(1, 0, 2).reshape(S, D)
            acc = part if acc is None else acc + part
        full[b] = acc
    return full


def _get_module():
    global _BUILT
    if _BUILT is None:
        _BUILT = build_module()
    return _BUILT


def _install_trace_shim():
    """This image's antenv lacks axon_hooks; provide the NTFF profile hook
    via ctypes so trace=True (or BASS_TRACE=1) works instead of crashing,
    and skip the artifact bucket upload."""
    try:
        import antenv.axon_hooks  # noqa: F401
        return
    except ImportError:
        pass
    import types
    import ctypes
    import contextlib

    so_path = "/opt/axon/libaxon_pjrt.so"
    mod = types.ModuleType("antenv.axon_hooks")
    try:
        lib = ctypes.CDLL(so_path)
        lib.axon_start_nrt_profile.argtypes = [
            ctypes.POINTER(ctypes.c_int64), ctypes.c_size_t]
        lib.axon_start_nrt_profile.restype = ctypes.c_int64
        lib.axon_stop_nrt_profile.argtypes = [ctypes.c_char_p]
        lib.axon_stop_nrt_profile.restype = ctypes.c_int64

        @contextlib.contextmanager
        def _hook(output_dir, device_ids):
            import jax
            jax.devices()
            if device_ids:
                ids = (ctypes.c_int64 * len(device_ids))(*device_ids)
                rc = lib.axon_start_nrt_profile(ids, len(device_ids))
            else:
                rc = lib.axon_start_nrt_profile(None, 0)
            if rc != 0:
                raise RuntimeError(f"axon_start_nrt_profile rc={rc}")
            try:
                yield
            finally:
                lib.axon_stop_nrt_profile(str(output_dir).encode())

        mod.get_axon_ntff_profile_hook = lambda: _hook
    except OSError:
        mod.get_axon_ntff_profile_hook = lambda: None
    mod.set_axon_ntff_profile_hook = lambda h: None
    sys.modules["antenv.axon_hooks"] = mod

    from concourse import bass_utils
    bass_utils.upload_artifacts = lambda tmpdir: tmpdir


def run_on_hw(in_maps, trace=False, trace_cores=None):
    _install_trace_shim()
    from concourse import bass_utils
    nc = _get_module()
    return bass_utils.run_bass_kernel_spmd(
        nc, in_maps, core_ids=list(range(N_CORES)),
        trace=trace, trace_cores=trace_cores)


def kernel(x, wq, wk, wv, wo):
    x = np.asarray(x, dtype=np.float32)
    wq = np.asarray(wq, dtype=np.float32)
    wk = np.asarray(wk, dtype=np.float32)
    wv = np.asarray(wv, dtype=np.float32)
    wo = np.asarray(wo, dtype=np.float32)
    in_maps = make_in_maps(x, wq, wk, wv, wo)
    res = run_on_hw(in_maps, trace=False)
    return assemble_output(res.results)
